# revision 5
# baseline (speedup 1.0000x reference)
"""Trainium2 Bass kernel for nn_Chambers (6-tower MLP + coupled sigmoid recurrence).

Data-parallel over 8 NeuronCores: each core processes a 16384-sample shard in
16 chunks of 1024 samples. res arrives pre-transposed per chunk via strided
DMA ([100, 1024] tiles, d on partitions) so the PE does no transposes and the
DVE no copies. The 4 MLP layers run as fp32r matmuls (full PE rate at N=512)
with chambers packed block-diagonally: L2/L3 pack chamber pairs into 128
partition rows via shifted zero-padded stationaries. L3 pair 2 is
double-packed across chunk pairs (rows 0:64 = even chunk, 64:128 = odd chunk)
so its silu runs once per two chunks. L4 accumulates all 16 chunks into a
persistent [96, 1024] PSUM tile using sliding-window stationary bands whose
nonzero columns land at rows 6i+c; the sigmoid recurrence and the raw output
read that tile directly, so no per-chunk raw copies exist.

Sync discipline: at most 1 sem wait + 1 update per engine instruction.
"Touch" ops (tiny matmul / 1-elem activation / copy) pre-observe cross-engine
sems; PE touches write into the about-to-be-started psum tile (the start=True
matmul re-zeroes the cells), so no scratch psum bank is needed. PSUM budget:
2 rotation tags x [128,1024] (4 banks) + pc double-pack tile (2) + praw (2).
"""
import numpy as np

import concourse.bass as bass
import concourse.mybir as mybir
from concourse.bass_utils import run_bass_kernel_spmd
from concourse.tile import TileContext
from concourse.tile_scheduler import N_PROCS
from concourse.vector_clock import ScopedClock
from bass_rust import add_dep_helper

F32 = mybir.dt.float32
F32R = mybir.dt.float32r
AF = mybir.ActivationFunctionType
ALU = mybir.AluOpType

B = 131072
NCORES = 8
BS = B // NCORES           # 16384 samples per core
T = 1024                   # chunk (samples)
NCH = BS // T              # 16 chunks
RES_DIM = 100
CF_ITERS = 5
CF_K = 0.02

# wf (fp32) column layout
B1C = 0        # 6 cols: b1 per chamber
B2PC = 6       # 3 cols: pair-packed b2 (rows 0:64 even, 64:128 odd)
B3PC = 9       # 1 col: merged pairs 0,1 b3 (rows 32c..32c+32 = b3[c])
B3DC = 10      # 1 col: double-packed pair2 b3 (b3[4],b3[5],b3[4],b3[5] by 32s)
B4C = 11       # 1 col: b4 tiled x16 over 96 rows
FCOLS = 12

# wr (fp32r) column layout
W1C = 0                    # 6*128
W2EC = 768                 # 3*64  (even chambers)
W2OC = 960                 # 3*128 (odd chambers shifted to out rows 64:127)
W3AC = 1344                # 128 (pair1 shifted to out rows 64:127)
W3BC0 = 1472               # 64  (pair0 -> out rows 0:63)
W3E = 1536                 # 128 (pair2, even chunk -> rows 0:63, rest zero)
W3O = 1664                 # 128 (pair2, odd chunk -> rows 64:127, rest zero)
CDC = 1792                 # 96 (block-diag decay*coupling*k per 6-row group)
I96C = 1888                # 96 identity
W4AB = 1984                # 186-col sliding band: window +90-6i gives cols 6i+c
W4BB = 2170                # 180-col sliding band: window +84-12p gives pair cols
RCOLS = 2350


class TC(TileContext):
    """TileContext with a walrus-compatible epilogue (split final waits)."""

    def _drain_and_barrier(self, tick_clock, wait_clock):
        nc = self.nc
        full = ScopedClock({None: tick_clock.global_clock})
        for scope, vc in full.items():
            for proc in range(N_PROCS):
                t = vc.peek_next(proc) - 1
                if t > 0:
                    sc = ScopedClock()
                    sc.require_at_least(scope, proc, t)
                    w = nc.sync.nop(nofuse=True)
                    wait_clock.add_sem_waits(w.ins, sc)
        for eng in nc.engines.values():
            eng.drain(fusable=False)
        nc.all_engine_barrier(sem_only=True)
        assert self.sems is not None
        popped = nc._tile_sem_poison_stack.pop()
        assert popped is self._sem_poison
        nc.clear_and_free_semaphores(list(self.sems.allocated().values()))
        for eng in nc.engines.values():
            eng.drain(fusable=False)
        nc.all_engine_barrier(sem_only=True)


def _order(after_inst, before_inst):
    if after_inst is not None and before_inst is not None:
        add_dep_helper(after_inst.ins, before_inst.ins, sync=False, reason="order")


def build_module():
    nc = bass.Bass()
    res_d = nc.dram_tensor("res", [BS, RES_DIM], F32R, kind="ExternalInput")
    wf_d = nc.dram_tensor("wf", [128, FCOLS], F32, kind="ExternalInput")
    wr_d = nc.dram_tensor("wr", [128, RCOLS], F32R, kind="ExternalInput")
    raw_d = nc.dram_tensor("raw_out", [96, T], F32, kind="ExternalOutput")
    act_d = nc.dram_tensor("act_out", [96, T], F32, kind="ExternalOutput")

    with TC(nc) as tc:
        with (
            tc.tile_pool(name="wconst", bufs=1) as wpool,
            tc.tile_pool(name="sbrt", bufs=4) as sbrt,
            tc.tile_pool(name="sbh", bufs=2) as sbh,
            tc.tile_pool(name="sbrec", bufs=1) as sbrec,
            tc.tile_pool(name="psmm", bufs=2, space="PSUM") as psmm,
            tc.tile_pool(name="pspc", bufs=1, space="PSUM") as pspc,
            tc.tile_pool(name="psraw", bufs=1, space="PSUM") as psraw,
        ):
            # DMA issue order: chunk-0 res first so compute starts early,
            # then weights, then later chunks stream behind.
            rt_tiles = {}

            def issue_rt(i):
                rt = sbrt.tile([100, T], F32R, tag="rT", bufs=4, name="rT")
                nc.sync.dma_start(
                    out=rt[:],
                    in_=res_d[i * T:(i + 1) * T].rearrange("b d -> d b"))
                rt_tiles[i] = rt

            issue_rt(0)
            wf = wpool.tile([128, FCOLS], F32)
            nc.sync.dma_start(out=wf[:], in_=wf_d[:])
            wr = wpool.tile([128, RCOLS], F32R)
            nc.sync.dma_start(out=wr[:, 0:W3O], in_=wr_d[:, 0:W3O])
            issue_rt(1)
            nc.sync.dma_start(out=wr[:, W3O:], in_=wr_d[:, W3O:])
            issue_rt(2)

            raw_sb = sbrec.tile([96, T], F32)
            act_r = sbrec.tile([96, T], F32R)
            act_o = sbrec.tile([96, T], F32)
            raw_r = sbrec.tile([96, T], F32R)
            scr = sbrec.tile([1, 4], F32)
            scrA = sbrec.tile([1, 512], F32)

            praw = psraw.tile([96, T], F32, name="praw")

            # ACT touch op: observe the wf DMA lane before first bias use
            nc.scalar.activation(scr[0:1, 0:1], wf[0:1, B1C:B1C + 1], AF.Copy)

            # Rolling state. Rule: each instruction carries at most one sem
            # wait; every cross-engine dependency is pre-observed by a real
            # "touch" instruction. PE touches write into the new psum tile's
            # first cells, which the subsequent start=True matmul re-zeroes.
            pe_tail = None
            act_tail = None
            dve_tail = None
            acol = [0]

            def pe_touch_into(dst_tile, cell, src_ap):
                nonlocal pe_tail
                m = nc.tensor.matmul(dst_tile[0:1, 2 * cell:2 * cell + 2],
                                     src_ap[:, 0:1], src_ap[:, 0:2],
                                     start=True, stop=True)
                _order(m, pe_tail)
                pe_tail = m
                return m

            def act_touch(src_ap):
                nonlocal act_tail
                t = acol[0]; acol[0] += 1
                assert t < 512
                s = nc.scalar.activation(scrA[0:1, t:t + 1], src_ap, AF.Copy)
                _order(s, act_tail)
                act_tail = s
                return s

            tag_rr = [0]
            tag_state = [None, None]

            def new_mm_tile(name, extra_srcs=(), width=T):
                tg = tag_rr[0] % 2
                tag_rr[0] += 1
                t = psmm.tile([128, width], F32, tag=f"mm{tg}", bufs=1,
                              name=name)
                cell = 0
                st = tag_state[tg]
                if st is not None:
                    tile_, row_, col_ = st
                    pe_touch_into(t, cell, tile_[row_:row_ + 1, col_:col_ + 2])
                    cell += 1
                    tag_state[tg] = None
                for src in extra_srcs:
                    pe_touch_into(t, cell, src)
                    cell += 1
                return t, tg

            def mm(out_ap, lhs_ap, rhs_ap, **kw):
                nonlocal pe_tail
                m = nc.tensor.matmul(out_ap, lhs_ap, rhs_ap, **kw)
                _order(m, pe_tail)
                pe_tail = m
                return m

            def silu(out_ap, pm_ap, bias_ap, out_tile, tg, func=AF.Silu):
                nonlocal act_tail
                s = nc.scalar.activation(out_ap, pm_ap, func, bias=bias_ap)
                _order(s, act_tail)
                act_tail = s
                if tg is not None:
                    tag_state[tg] = (out_tile, 0, 0)
                return s

            # deferred emitters keyed by chunk
            h3a_tiles = {}
            h3b_tiles = {}
            pc_state = [None]  # live pc tile for the current chunk pair

            def emit_l4a(j):
                # praw[6j+c] += W4[c] . h3a_j via sliding-window band
                h3a = h3a_tiles.pop(j)
                if j == 0:
                    # first praw write: touch cells land in the start region
                    # (the start=True matmul re-zeroes them); they observe the
                    # wr tail-segment DMA lane and the h3a silu. For j>0 the
                    # matmul itself carries the single silu wait (or it is
                    # already observed via rotation touches).
                    pe_touch_into(praw, 0, wr[0:1, W4AB:W4AB + 2])
                    pe_touch_into(praw, 1, h3a[0:1, 0:2])
                st = wr[:, W4AB + 90 - 6 * j:W4AB + 186 - 6 * j]
                for s in range(2):
                    mm(praw[0:96, s * 512:(s + 1) * 512], st,
                       h3a[:, s * 512:(s + 1) * 512],
                       start=(j == 0), stop=False)

            def emit_l4b(p, last=False):
                h3b = h3b_tiles.pop(p)
                st = wr[:, W4BB + 84 - 12 * p:W4BB + 180 - 12 * p]
                for s in range(2):
                    mm(praw[0:96, s * 512:(s + 1) * 512], st,
                       h3b[:, s * 512:(s + 1) * 512],
                       start=False, stop=(last and s == 1))

            def emit_pc(j):
                # L3 pair 2, double-packed across (even, odd) chunk pairs.
                h2_2 = h2_pair2.pop(j)
                if j % 2 == 0:
                    pc = pspc.tile([128, T], F32, tag="pc", bufs=1, name="pc")
                    if pc_state[0] is not None:
                        pe_touch_into(pc, 0, pc_state[0][0:1, 0:2])
                    pc_state[0] = pc
                    for s in range(2):
                        mm(pc[:, s * 512:(s + 1) * 512],
                           wr[:, W3E:W3E + 128],
                           h2_2[:, s * 512:(s + 1) * 512],
                           start=True, stop=False)
                else:
                    pc = pc_state[0]
                    for s in range(2):
                        mm(pc[:, s * 512:(s + 1) * 512],
                           wr[:, W3O:W3O + 128],
                           h2_2[:, s * 512:(s + 1) * 512],
                           start=False, stop=True)
                    h3b = sbh.tile([128, T], F32R, tag="h3b", bufs=2,
                                   name="h3b")
                    act_touch(pc[0:1, 512:513])
                    silu(h3b[:], pc[:], wf[:, B3DC:B3DC + 1], h3b, None)
                    pc_state[0] = h3b  # next pair's touch source
                    h3b_tiles[(j - 1) // 2] = h3b

            h2_pair2 = {}

            for i in range(NCH):
                rT = rt_tiles.pop(i)
                if i + 3 < NCH:
                    issue_rt(i + 3)

                # L1: 3 chamber-pairs
                h1s = []
                for cp in range(3):
                    ha = sbh.tile([128, T], F32R, tag="h1", bufs=7, name="h1a")
                    hb = sbh.tile([128, T], F32R, tag="h1", bufs=7, name="h1b")
                    extras = [rT[0:1, 0:2]] if cp == 0 else []
                    if i == 0 and cp == 0:
                        extras.append(wr[0:1, 0:2])
                    pa, ta = new_mm_tile("pm1a", extra_srcs=extras)
                    pb, tb = new_mm_tile("pm1b")
                    for s in range(2):
                        mm(pa[:, s * 512:(s + 1) * 512],
                           wr[0:100, W1C + 2 * cp * 128:W1C + (2 * cp + 1) * 128],
                           rT[:, s * 512:(s + 1) * 512], start=True, stop=True)
                    for s in range(2):
                        mm(pb[:, s * 512:(s + 1) * 512],
                           wr[0:100, W1C + (2 * cp + 1) * 128:W1C + (2 * cp + 2) * 128],
                           rT[:, s * 512:(s + 1) * 512], start=True, stop=True)
                    act_touch(pb[0:1, 512:513])
                    silu(ha[:], pa[:], wf[:, B1C + 2 * cp:B1C + 2 * cp + 1],
                         ha, ta)
                    silu(hb[:], pb[:], wf[:, B1C + 2 * cp + 1:B1C + 2 * cp + 2],
                         hb, tb)
                    h1s.extend([ha, hb])
                    if cp == 1 and i >= 1:
                        emit_l4a(i - 1)
                        if i >= 3 and i % 2 == 1:
                            emit_l4b((i - 3) // 2)
                    if cp == 2 and i >= 1:
                        emit_pc(i - 1)

                # L2: per pair, odd chamber shifted to rows 64:127
                h2s = []
                l2t = []
                for pr in range(3):
                    pm2, tg2 = new_mm_tile("pm2")
                    for s in range(2):
                        mm(pm2[:, s * 512:(s + 1) * 512],
                           wr[:, W2OC + pr * 128:W2OC + (pr + 1) * 128],
                           h1s[2 * pr + 1][:, s * 512:(s + 1) * 512],
                           start=True, stop=False)
                        mm(pm2[0:64, s * 512:(s + 1) * 512],
                           wr[:, W2EC + pr * 64:W2EC + (pr + 1) * 64],
                           h1s[2 * pr][:, s * 512:(s + 1) * 512],
                           start=False, stop=True)
                    l2t.append((pm2, tg2))
                for pr in range(3):
                    pm2, tg2 = l2t[pr]
                    if pr == 0:
                        act_touch(pm2[0:1, 512:513])
                    h2 = sbh.tile([128, T], F32R, tag="h2", bufs=4, name="h2")
                    silu(h2[:], pm2[:], wf[:, B2PC + pr:B2PC + pr + 1],
                         h2, tg2)
                    h2s.append(h2)
                h2_pair2[i] = h2s[2]

                # L3 pairs 0,1 merged into one tile
                h3a = sbh.tile([128, T], F32R, tag="h3a", bufs=2, name="h3a")
                pa3, ta3 = new_mm_tile("pm3")
                for s in range(2):
                    mm(pa3[:, s * 512:(s + 1) * 512],
                       wr[:, W3AC:W3AC + 128],
                       h2s[1][:, s * 512:(s + 1) * 512], start=True, stop=False)
                    mm(pa3[0:64, s * 512:(s + 1) * 512],
                       wr[:, W3BC0:W3BC0 + 64],
                       h2s[0][:, s * 512:(s + 1) * 512], start=False, stop=True)
                act_touch(pa3[0:1, 512:513])
                silu(h3a[:], pa3[:], wf[:, B3PC:B3PC + 1], h3a, ta3)
                h3a_tiles[i] = h3a

            # drain deferred tail work
            emit_l4a(NCH - 1)
            emit_pc(NCH - 1)
            emit_l4b(7, last=True)

            # ---- coupled sigmoid recurrence on praw [96, T] ----
            b4ap = wf[0:96, B4C:B4C + 1]
            act_touch(praw[0:1, 0:1])
            sig = nc.scalar.activation(act_r[:], praw[0:96, :], AF.Sigmoid,
                                       bias=b4ap)
            _order(sig, act_tail)
            act_tail = sig
            rb = nc.scalar.activation(raw_sb[:], praw[0:96, :], AF.Identity,
                                      bias=b4ap)
            _order(rb, act_tail)
            act_tail = rb
            nc.sync.dma_start(out=raw_d[:], in_=raw_sb[:])
            c0 = nc.vector.tensor_copy(scr[0:1, 2:3], raw_sb[0:1, 0:1])
            dve_tail = c0
            cpr = nc.vector.tensor_copy(raw_r[:], raw_sb[:])
            _order(cpr, dve_tail)
            dve_tail = cpr

            for kk in range(CF_ITERS):
                dst = act_r if kk < CF_ITERS - 1 else act_o
                for s in range(2):
                    extras = []
                    if s == 0:
                        # observe the latest sigmoid (kk-1 s=1) and, on the
                        # first iter, the DVE raw_r copy
                        extras.append(act_r[0:1, 0:2])
                        if kk == 0:
                            extras.append(raw_r[0:1, 0:2])
                    pm5, tg5 = new_mm_tile("pm5", extra_srcs=extras,
                                           width=512)
                    mm(pm5[0:96, 0:512],
                       wr[0:96, CDC:CDC + 96],
                       act_r[:, s * 512:(s + 1) * 512],
                       start=True, stop=False)
                    mm(pm5[0:96, 0:512],
                       wr[0:96, I96C:I96C + 96],
                       raw_r[:, s * 512:(s + 1) * 512],
                       start=False, stop=True)
                    act_touch(pm5[0:1, 0:1])
                    sg = nc.scalar.activation(
                        dst[:, s * 512:(s + 1) * 512], pm5[0:96, 0:512],
                        AF.Sigmoid)
                    _order(sg, act_tail)
                    act_tail = sg
                    tag_state[tg5] = (dst, 0, s * 512)

            nc.sync.dma_start(out=act_d[:], in_=act_o[:])

    return nc


def _pack_consts(W1, b1, W2, b2, W3, b3, W4, b4, coupling, decay):
    wf = np.zeros((128, FCOLS), dtype=np.float32)
    for c in range(6):
        wf[:, B1C + c] = b1[c]
    for pr in range(3):
        wf[0:64, B2PC + pr] = b2[2 * pr]
        wf[64:128, B2PC + pr] = b2[2 * pr + 1]
    for c in range(4):
        wf[c * 32:(c + 1) * 32, B3PC] = b3[c]
    wf[0:32, B3DC] = b3[4]
    wf[32:64, B3DC] = b3[5]
    wf[64:96, B3DC] = b3[4]
    wf[96:128, B3DC] = b3[5]
    wf[0:96, B4C] = np.tile(b4, NCH)

    wr = np.zeros((128, RCOLS), dtype=np.float32)
    for c in range(6):
        wr[0:100, W1C + c * 128:W1C + (c + 1) * 128] = W1[c]
    for pr in range(3):
        wr[:, W2EC + pr * 64:W2EC + (pr + 1) * 64] = W2[2 * pr]
        wr[:, W2OC + pr * 128 + 64:W2OC + (pr + 1) * 128] = W2[2 * pr + 1]
    # L3 merged pairs 0,1: pair1 shifted to out rows 64:127
    wr[0:64, W3AC + 64:W3AC + 96] = W3[2]
    wr[64:128, W3AC + 96:W3AC + 128] = W3[3]
    wr[0:64, W3BC0:W3BC0 + 32] = W3[0]
    wr[64:128, W3BC0 + 32:W3BC0 + 64] = W3[1]
    # L3 pair 2 double-pack: even chunk -> rows 0:63, odd chunk -> 64:127
    wr[0:64, W3E:W3E + 32] = W3[4]
    wr[64:128, W3E + 32:W3E + 64] = W3[5]
    wr[0:64, W3O + 64:W3O + 96] = W3[4]
    wr[64:128, W3O + 96:W3O + 128] = W3[5]
    cd = (decay[:, None] * coupling * CF_K).astype(np.float32)
    for g in range(16):
        wr[6 * g:6 * g + 6, CDC + 6 * g:CDC + 6 * g + 6] = cd
    wr[0:96, I96C:I96C + 96] = np.eye(96, dtype=np.float32)
    # W4 sliding bands
    for c in range(4):
        wr[c * 32:(c + 1) * 32, W4AB + 90 + c] = W4[c]
    wr[0:32, W4BB + 88] = W4[4]
    wr[32:64, W4BB + 89] = W4[5]
    wr[64:96, W4BB + 94] = W4[4]
    wr[96:128, W4BB + 95] = W4[5]
    return wf, wr


def _unshard(per_core, key):
    """[96, T] layout (row 6i+c, col j = sample i*T+j) -> [B, 6]."""
    outs = []
    for r in per_core:
        a = r[key].reshape(NCH, 6, T)
        outs.append(a.transpose(0, 2, 1).reshape(BS, 6))
    return np.concatenate(outs, axis=0)


def kernel(res, W1, b1, W2, b2, W3, b3, W4, b4, coupling, decay):
    res = np.asarray(res, dtype=np.float32)
    args = [np.asarray(a, dtype=np.float32)
            for a in (W1, b1, W2, b2, W3, b3, W4, b4, coupling, decay)]
    wf, wr = _pack_consts(*args)

    nc = build_module()
    in_maps = [
        {"res": np.ascontiguousarray(res[i * BS:(i + 1) * BS]),
         "wf": wf, "wr": wr}
        for i in range(NCORES)
    ]
    results = run_bass_kernel_spmd(nc, in_maps, core_ids=list(range(NCORES)))
    act = _unshard(results.results, "act_out")
    raw = _unshard(results.results, "raw_out")
    return act, raw


# revision 8
# speedup vs baseline: 2.7468x; 2.7468x over previous
"""Trainium2 Bass kernel for nn_Chambers (6-tower MLP + coupled sigmoid recurrence).

Data-parallel over 8 NeuronCores: each core processes a 16384-sample shard in
16 chunks of 1024 samples. res tiles are PE-transposed (fp32, exact) into
[100, 1024] activation tiles; the transposes land in rotation PSUM tiles so
no dedicated transpose bank exists. The 4 MLP layers run as fp32r matmuls
(full PE rate at N=512) with chamber pairs packed into 128 partition rows via
shifted zero-padded stationaries. L3 pair 2 is double-packed across chunk
pairs (rows 0:64 = even chunk, 64:128 = odd chunk) so its silu runs once per
two chunks. L4 accumulates all 16 chunks into a persistent [96, 1024] PSUM
tile using sliding-window stationary bands whose nonzero columns land at rows
6i+c; the sigmoid recurrence and the raw output read that tile directly, so
no per-chunk raw copies exist.

Sync discipline: at most 1 sem wait + 1 update per engine instruction.
"Touch" ops (tiny matmul / 1-elem activation / 1-elem copy) pre-observe
cross-engine sems; PE touches write into the about-to-be-started psum tile
(the start=True matmul or transpose re-zeroes the cells), so no scratch psum
bank is needed. PSUM budget: 2 rotation tags x [128,1024] (4 banks) + pc
double-pack tile (2) + praw (2) = 8 banks.
"""
import numpy as np

import concourse.bass as bass
import concourse.mybir as mybir
from concourse.bass_utils import run_bass_kernel_spmd
from concourse.tile import TileContext
from concourse.tile_scheduler import N_PROCS
from concourse.vector_clock import ScopedClock
from bass_rust import add_dep_helper

F32 = mybir.dt.float32
F32R = mybir.dt.float32r
AF = mybir.ActivationFunctionType
ALU = mybir.AluOpType

B = 131072
NCORES = 8
BS = B // NCORES           # 16384 samples per core
T = 1024                   # chunk (samples)
NCH = BS // T              # 16 chunks
RES_DIM = 100
CF_ITERS = 5
CF_K = 0.02

# wf (fp32) column layout
IDC = 0        # identity [128,128] for PE transposes
B1C = 128      # 6 cols: b1 per chamber
B2PC = 134     # 3 cols: pair-packed b2 (rows 0:64 even, 64:128 odd)
B3PC = 137     # 1 col: merged pairs 0,1 b3 (rows 32c..32c+32 = b3[c])
B3DC = 138     # 1 col: double-packed pair2 b3 (b3[4],b3[5],b3[4],b3[5])
B4C = 139      # 1 col: b4 tiled x16 over 96 rows
FCOLS = 140

# wr (fp32r) column layout
W1C = 0                    # 6*128
W2EC = 768                 # 3*64  (even chambers)
W2OC = 960                 # 3*128 (odd chambers shifted to out rows 64:127)
W3AC = 1344                # 128 (pair1 shifted to out rows 64:127)
W3BC0 = 1472               # 64  (pair0 -> out rows 0:63)
W3E = 1536                 # 128 (pair2, even chunk -> rows 0:63, rest zero)
W3O = 1664                 # 128 (pair2, odd chunk -> rows 64:127, rest zero)
CDC = 1792                 # 96 (block-diag decay*coupling*k per 6-row group)
I96C = 1888                # 96 identity
W4AB = 1984                # 186-col sliding band: window +90-6i -> cols 6i+c
W4BB = 2170                # 180-col sliding band: window +84-12p -> pair cols
RCOLS = 2350


class TC(TileContext):
    """TileContext with a walrus-compatible epilogue (split final waits)."""

    def _drain_and_barrier(self, tick_clock, wait_clock):
        nc = self.nc
        full = ScopedClock({None: tick_clock.global_clock})
        for scope, vc in full.items():
            for proc in range(N_PROCS):
                t = vc.peek_next(proc) - 1
                if t > 0:
                    sc = ScopedClock()
                    sc.require_at_least(scope, proc, t)
                    w = nc.sync.nop(nofuse=True)
                    wait_clock.add_sem_waits(w.ins, sc)
        for eng in nc.engines.values():
            eng.drain(fusable=False)
        nc.all_engine_barrier(sem_only=True)
        assert self.sems is not None
        popped = nc._tile_sem_poison_stack.pop()
        assert popped is self._sem_poison
        nc.clear_and_free_semaphores(list(self.sems.allocated().values()))
        for eng in nc.engines.values():
            eng.drain(fusable=False)
        nc.all_engine_barrier(sem_only=True)


def _order(after_inst, before_inst):
    if after_inst is not None and before_inst is not None:
        add_dep_helper(after_inst.ins, before_inst.ins, sync=False, reason="order")


def build_module():
    nc = bass.Bass()
    res_d = nc.dram_tensor("res", [BS, RES_DIM], F32, kind="ExternalInput")
    wf_d = nc.dram_tensor("wf", [128, FCOLS], F32, kind="ExternalInput")
    wr_d = nc.dram_tensor("wr", [128, RCOLS], F32R, kind="ExternalInput")
    raw_d = nc.dram_tensor("raw_out", [96, T], F32, kind="ExternalOutput")
    act_d = nc.dram_tensor("act_out", [96, T], F32, kind="ExternalOutput")

    with TC(nc) as tc:
        with (
            tc.tile_pool(name="wconst", bufs=1) as wpool,
            tc.tile_pool(name="sbrt", bufs=3) as sbrt,
            tc.tile_pool(name="sbh", bufs=2) as sbh,
            tc.tile_pool(name="sbrec", bufs=1) as sbrec,
            tc.tile_pool(name="psmm", bufs=2, space="PSUM") as psmm,
            tc.tile_pool(name="pspc", bufs=1, space="PSUM") as pspc,
            tc.tile_pool(name="psraw", bufs=1, space="PSUM") as psraw,
        ):
            # DMA issue order: chunk-0 res and L1 weights first so compute
            # starts early; the bulky remainder streams behind.
            res_sb0 = wpool.tile([128, 8 * RES_DIM], F32)
            nc.sync.dma_start(
                out=res_sb0[:],
                in_=res_d[0:T].rearrange("(p n) d -> p (n d)", p=128))
            wf = wpool.tile([128, FCOLS], F32)
            nc.sync.dma_start(out=wf[:], in_=wf_d[:])
            wr = wpool.tile([128, RCOLS], F32R)
            nc.sync.dma_start(out=wr[:, 0:W3O], in_=wr_d[:, 0:W3O])
            res_sb1 = wpool.tile([128, 3 * 8 * RES_DIM], F32)
            nc.sync.dma_start(
                out=res_sb1[:],
                in_=res_d[T:4 * T].rearrange("(p n) d -> p (n d)", p=128))
            nc.sync.dma_start(out=wr[:, W3O:], in_=wr_d[:, W3O:])
            res_sb2 = wpool.tile([128, (NCH - 4) * 8 * RES_DIM], F32)
            nc.sync.dma_start(
                out=res_sb2[:],
                in_=res_d[4 * T:].rearrange("(p n) d -> p (n d)", p=128))
            ident = wf[:, IDC:IDC + 128]

            raw_sb = sbrec.tile([96, T], F32)
            act_r = sbrec.tile([96, T], F32R)
            act_o = sbrec.tile([96, T], F32)
            raw_r = sbrec.tile([96, T], F32R)
            scr = sbrec.tile([1, 4], F32)
            scrA = sbrec.tile([1, 512], F32)
            scrD = sbrec.tile([1, 64], F32)

            praw = psraw.tile([96, T], F32, name="praw")

            # ACT touch op: observe the wf DMA lane before first bias use
            nc.scalar.activation(scr[0:1, 0:1], wf[0:1, B1C:B1C + 1], AF.Copy)

            # Rolling state. Rule: each instruction carries at most one sem
            # wait; every cross-engine dependency is pre-observed by a real
            # "touch" instruction. PE touches write into the new psum tile's
            # first cells, which the subsequent start=True matmul (or
            # transpose) re-zeroes.
            pe_tail = None
            act_tail = None
            dve_tail = None
            acol = [0]
            dcol = [0]

            def pe_touch_into(dst_tile, cell, src_ap):
                nonlocal pe_tail
                m = nc.tensor.matmul(dst_tile[0:1, 2 * cell:2 * cell + 2],
                                     src_ap[:, 0:1], src_ap[:, 0:2],
                                     start=True, stop=True)
                _order(m, pe_tail)
                pe_tail = m
                return m

            def act_touch(src_ap):
                nonlocal act_tail
                t = acol[0]; acol[0] += 1
                assert t < 512
                s = nc.scalar.activation(scrA[0:1, t:t + 1], src_ap, AF.Copy)
                _order(s, act_tail)
                act_tail = s
                return s

            def dve_touch(src_ap):
                nonlocal dve_tail
                t = dcol[0]; dcol[0] += 1
                assert t < 64
                c = nc.vector.tensor_copy(scrD[0:1, t:t + 1], src_ap)
                _order(c, dve_tail)
                dve_tail = c
                return c

            tag_rr = [0]
            tag_state = [None, None]

            def new_mm_tile(name, extra_srcs=(), width=T):
                tg = tag_rr[0] % 2
                tag_rr[0] += 1
                t = psmm.tile([128, width], F32, tag=f"mm{tg}", bufs=1,
                              name=name)
                cell = 0
                st = tag_state[tg]
                if st is not None:
                    tile_, row_, col_ = st
                    pe_touch_into(t, cell, tile_[row_:row_ + 1, col_:col_ + 2])
                    cell += 1
                    tag_state[tg] = None
                for src in extra_srcs:
                    pe_touch_into(t, cell, src)
                    cell += 1
                return t, tg

            def mm(out_ap, lhs_ap, rhs_ap, **kw):
                nonlocal pe_tail
                m = nc.tensor.matmul(out_ap, lhs_ap, rhs_ap, **kw)
                _order(m, pe_tail)
                pe_tail = m
                return m

            def silu(out_ap, pm_ap, bias_ap, out_tile, tg, func=AF.Silu):
                nonlocal act_tail
                s = nc.scalar.activation(out_ap, pm_ap, func, bias=bias_ap)
                _order(s, act_tail)
                act_tail = s
                if tg is not None:
                    tag_state[tg] = (out_tile, 0, 0)
                return s

            rt_tiles = {}
            h3a_tiles = {}
            h3b_tiles = {}
            h2_pair2 = {}
            pc_state = [None]  # WAR source for the pc psum slot

            def emit_tr(i):
                """Transpose chunk i's res into a [100, T] f32r SBUF tile via
                a rotation psum tile."""
                nonlocal pe_tail, dve_tail
                if i == 0:
                    rq, coff = res_sb0, 0
                elif i < 4:
                    rq, coff = res_sb1, (i - 1) * 8 * RES_DIM
                else:
                    rq, coff = res_sb2, (i - 4) * 8 * RES_DIM
                extras = []
                if i in (0, 1, 4):
                    extras.append(rq[0:1, coff:coff + 2])
                if i == 0:
                    extras.append(wr[0:1, 0:2])
                ptr, tgp = new_mm_tile("ptr", extra_srcs=extras)
                for n in range(8):
                    t_ = nc.tensor.transpose(
                        ptr[0:100, n * 128:(n + 1) * 128],
                        rq[:, coff + n * RES_DIM:coff + (n + 1) * RES_DIM],
                        ident,
                    )
                    _order(t_, pe_tail)
                    pe_tail = t_
                rT = sbrt.tile([100, T], F32R, tag="rT", bufs=3, name="rT")
                dve_touch(ptr[0:1, 0:1])
                cp = nc.vector.tensor_copy(rT[:100, :], ptr[:100, :])
                _order(cp, dve_tail)
                dve_tail = cp
                tag_state[tgp] = (rT, 0, 0)
                rt_tiles[i] = rT

            def emit_l4a(j):
                # praw[6j+c] += W4[c] . h3a_j via sliding-window band
                h3a = h3a_tiles.pop(j)
                if j == 0:
                    # first praw write: touch cells land in the start region
                    # (the start=True matmul re-zeroes them); they observe
                    # the wr tail-segment DMA lane and the h3a silu. For j>0
                    # the matmul itself carries the single silu wait (or it
                    # is already observed via rotation touches).
                    pe_touch_into(praw, 0, wr[0:1, W4AB:W4AB + 2])
                    pe_touch_into(praw, 1, h3a[0:1, 0:2])
                st = wr[:, W4AB + 90 - 6 * j:W4AB + 186 - 6 * j]
                for s in range(2):
                    mm(praw[0:96, s * 512:(s + 1) * 512], st,
                       h3a[:, s * 512:(s + 1) * 512],
                       start=(j == 0), stop=False)

            def emit_l4b(p, last=False):
                h3b = h3b_tiles.pop(p)
                st = wr[:, W4BB + 84 - 12 * p:W4BB + 180 - 12 * p]
                for s in range(2):
                    mm(praw[0:96, s * 512:(s + 1) * 512], st,
                       h3b[:, s * 512:(s + 1) * 512],
                       start=False, stop=(last and s == 1))

            def emit_pc(j):
                # L3 pair 2, double-packed across (even, odd) chunk pairs.
                h2_2 = h2_pair2.pop(j)
                if j % 2 == 0:
                    pc = pspc.tile([128, T], F32, tag="pc", bufs=1, name="pc")
                    if pc_state[0] is not None:
                        pe_touch_into(pc, 0, pc_state[0][0:1, 0:2])
                    pc_state[0] = pc
                    for s in range(2):
                        mm(pc[:, s * 512:(s + 1) * 512],
                           wr[:, W3E:W3E + 128],
                           h2_2[:, s * 512:(s + 1) * 512],
                           start=True, stop=False)
                else:
                    pc = pc_state[0]
                    for s in range(2):
                        mm(pc[:, s * 512:(s + 1) * 512],
                           wr[:, W3O:W3O + 128],
                           h2_2[:, s * 512:(s + 1) * 512],
                           start=False, stop=True)
                    h3b = sbh.tile([128, T], F32R, tag="h3b", bufs=2,
                                   name="h3b")
                    act_touch(pc[0:1, 512:513])
                    silu(h3b[:], pc[:], wf[:, B3DC:B3DC + 1], h3b, None)
                    pc_state[0] = h3b  # next pair's WAR touch source
                    h3b_tiles[(j - 1) // 2] = h3b

            emit_tr(0)
            for i in range(NCH):
                rT = rt_tiles.pop(i)

                # L1: 3 chamber-pairs
                h1s = []
                for cp in range(3):
                    ha = sbh.tile([128, T], F32R, tag="h1", bufs=7, name="h1a")
                    hb = sbh.tile([128, T], F32R, tag="h1", bufs=7, name="h1b")
                    extras = [rT[0:1, 0:2]] if cp == 0 else []
                    pa, ta = new_mm_tile("pm1a", extra_srcs=extras)
                    pb, tb = new_mm_tile("pm1b")
                    for s in range(2):
                        mm(pa[:, s * 512:(s + 1) * 512],
                           wr[0:100, W1C + 2 * cp * 128:W1C + (2 * cp + 1) * 128],
                           rT[:, s * 512:(s + 1) * 512], start=True, stop=True)
                    for s in range(2):
                        mm(pb[:, s * 512:(s + 1) * 512],
                           wr[0:100, W1C + (2 * cp + 1) * 128:W1C + (2 * cp + 2) * 128],
                           rT[:, s * 512:(s + 1) * 512], start=True, stop=True)
                    act_touch(pb[0:1, 512:513])
                    silu(ha[:], pa[:], wf[:, B1C + 2 * cp:B1C + 2 * cp + 1],
                         ha, ta)
                    silu(hb[:], pb[:], wf[:, B1C + 2 * cp + 1:B1C + 2 * cp + 2],
                         hb, tb)
                    h1s.extend([ha, hb])
                    if cp == 1 and i >= 1:
                        emit_l4a(i - 1)
                        if i >= 3 and i % 2 == 1:
                            emit_l4b((i - 3) // 2)
                    if cp == 2 and i >= 1:
                        emit_pc(i - 1)

                if i + 1 < NCH:
                    emit_tr(i + 1)

                # L2: per pair, odd chamber shifted to rows 64:127
                h2s = []
                l2t = []
                for pr in range(3):
                    pm2, tg2 = new_mm_tile("pm2")
                    for s in range(2):
                        mm(pm2[:, s * 512:(s + 1) * 512],
                           wr[:, W2OC + pr * 128:W2OC + (pr + 1) * 128],
                           h1s[2 * pr + 1][:, s * 512:(s + 1) * 512],
                           start=True, stop=False)
                        mm(pm2[0:64, s * 512:(s + 1) * 512],
                           wr[:, W2EC + pr * 64:W2EC + (pr + 1) * 64],
                           h1s[2 * pr][:, s * 512:(s + 1) * 512],
                           start=False, stop=True)
                    l2t.append((pm2, tg2))
                for pr in range(3):
                    pm2, tg2 = l2t[pr]
                    if pr == 0:
                        act_touch(pm2[0:1, 512:513])
                    h2 = sbh.tile([128, T], F32R, tag="h2", bufs=4, name="h2")
                    silu(h2[:], pm2[:], wf[:, B2PC + pr:B2PC + pr + 1],
                         h2, tg2)
                    h2s.append(h2)
                h2_pair2[i] = h2s[2]

                # L3 pairs 0,1 merged into one tile
                h3a = sbh.tile([128, T], F32R, tag="h3a", bufs=2, name="h3a")
                pa3, ta3 = new_mm_tile("pm3")
                for s in range(2):
                    mm(pa3[:, s * 512:(s + 1) * 512],
                       wr[:, W3AC:W3AC + 128],
                       h2s[1][:, s * 512:(s + 1) * 512], start=True, stop=False)
                    mm(pa3[0:64, s * 512:(s + 1) * 512],
                       wr[:, W3BC0:W3BC0 + 64],
                       h2s[0][:, s * 512:(s + 1) * 512], start=False, stop=True)
                act_touch(pa3[0:1, 512:513])
                silu(h3a[:], pa3[:], wf[:, B3PC:B3PC + 1], h3a, ta3)
                h3a_tiles[i] = h3a

            # drain deferred tail work
            emit_l4a(NCH - 1)
            emit_pc(NCH - 1)
            emit_l4b(7, last=True)

            # ---- coupled sigmoid recurrence on praw [96, T] ----
            b4ap = wf[0:96, B4C:B4C + 1]
            act_touch(praw[0:1, 0:1])
            sig = nc.scalar.activation(act_r[:], praw[0:96, :], AF.Sigmoid,
                                       bias=b4ap)
            _order(sig, act_tail)
            act_tail = sig
            rb = nc.scalar.activation(raw_sb[:], praw[0:96, :], AF.Identity,
                                      bias=b4ap)
            _order(rb, act_tail)
            act_tail = rb
            nc.sync.dma_start(out=raw_d[:], in_=raw_sb[:])
            dve_touch(raw_sb[0:1, 0:1])
            cpr = nc.vector.tensor_copy(raw_r[:], raw_sb[:])
            _order(cpr, dve_tail)
            dve_tail = cpr

            for kk in range(CF_ITERS):
                dst = act_r if kk < CF_ITERS - 1 else act_o
                for s in range(2):
                    extras = []
                    if s == 0:
                        # observe the latest sigmoid (kk-1 s=1) and, on the
                        # first iter, the DVE raw_r copy
                        extras.append(act_r[0:1, 0:2])
                        if kk == 0:
                            extras.append(raw_r[0:1, 0:2])
                    pm5, tg5 = new_mm_tile("pm5", extra_srcs=extras,
                                           width=512)
                    mm(pm5[0:96, 0:512],
                       wr[0:96, CDC:CDC + 96],
                       act_r[:, s * 512:(s + 1) * 512],
                       start=True, stop=False)
                    mm(pm5[0:96, 0:512],
                       wr[0:96, I96C:I96C + 96],
                       raw_r[:, s * 512:(s + 1) * 512],
                       start=False, stop=True)
                    act_touch(pm5[0:1, 0:1])
                    sg = nc.scalar.activation(
                        dst[:, s * 512:(s + 1) * 512], pm5[0:96, 0:512],
                        AF.Sigmoid)
                    _order(sg, act_tail)
                    act_tail = sg
                    tag_state[tg5] = (dst, 0, s * 512)

            nc.sync.dma_start(out=act_d[:], in_=act_o[:])

    return nc


def _pack_consts(W1, b1, W2, b2, W3, b3, W4, b4, coupling, decay):
    wf = np.zeros((128, FCOLS), dtype=np.float32)
    wf[:, IDC:IDC + 128] = np.eye(128, dtype=np.float32)
    for c in range(6):
        wf[:, B1C + c] = b1[c]
    for pr in range(3):
        wf[0:64, B2PC + pr] = b2[2 * pr]
        wf[64:128, B2PC + pr] = b2[2 * pr + 1]
    for c in range(4):
        wf[c * 32:(c + 1) * 32, B3PC] = b3[c]
    wf[0:32, B3DC] = b3[4]
    wf[32:64, B3DC] = b3[5]
    wf[64:96, B3DC] = b3[4]
    wf[96:128, B3DC] = b3[5]
    wf[0:96, B4C] = np.tile(b4, NCH)

    wr = np.zeros((128, RCOLS), dtype=np.float32)
    for c in range(6):
        wr[0:100, W1C + c * 128:W1C + (c + 1) * 128] = W1[c]
    for pr in range(3):
        wr[:, W2EC + pr * 64:W2EC + (pr + 1) * 64] = W2[2 * pr]
        wr[:, W2OC + pr * 128 + 64:W2OC + (pr + 1) * 128] = W2[2 * pr + 1]
    # L3 merged pairs 0,1: pair1 shifted to out rows 64:127
    wr[0:64, W3AC + 64:W3AC + 96] = W3[2]
    wr[64:128, W3AC + 96:W3AC + 128] = W3[3]
    wr[0:64, W3BC0:W3BC0 + 32] = W3[0]
    wr[64:128, W3BC0 + 32:W3BC0 + 64] = W3[1]
    # L3 pair 2 double-pack: even chunk -> rows 0:63, odd chunk -> 64:127
    wr[0:64, W3E:W3E + 32] = W3[4]
    wr[64:128, W3E + 32:W3E + 64] = W3[5]
    wr[0:64, W3O + 64:W3O + 96] = W3[4]
    wr[64:128, W3O + 96:W3O + 128] = W3[5]
    cd = (decay[:, None] * coupling * CF_K).astype(np.float32)
    for g in range(16):
        wr[6 * g:6 * g + 6, CDC + 6 * g:CDC + 6 * g + 6] = cd
    wr[0:96, I96C:I96C + 96] = np.eye(96, dtype=np.float32)
    # W4 sliding bands
    for c in range(4):
        wr[c * 32:(c + 1) * 32, W4AB + 90 + c] = W4[c]
    wr[0:32, W4BB + 88] = W4[4]
    wr[32:64, W4BB + 89] = W4[5]
    wr[64:96, W4BB + 94] = W4[4]
    wr[96:128, W4BB + 95] = W4[5]
    return wf, wr


def _unshard(per_core, key):
    """[96, T] layout (row 6i+c, col j) -> [BS, 6] per core, concat.

    Col j of chunk i: n = j//128, p = j%128 maps to sample i*T + 8p + n
    (p-major transpose block order)."""
    outs = []
    for r in per_core:
        a = r[key].reshape(NCH, 6, 8, 128)         # [i, c, n, p]
        out = a.transpose(0, 3, 2, 1).reshape(BS, 6)
        outs.append(out)
    return np.concatenate(outs, axis=0)


def kernel(res, W1, b1, W2, b2, W3, b3, W4, b4, coupling, decay):
    res = np.asarray(res, dtype=np.float32)
    args = [np.asarray(a, dtype=np.float32)
            for a in (W1, b1, W2, b2, W3, b3, W4, b4, coupling, decay)]
    wf, wr = _pack_consts(*args)

    nc = build_module()
    in_maps = [
        {"res": np.ascontiguousarray(res[i * BS:(i + 1) * BS]),
         "wf": wf, "wr": wr}
        for i in range(NCORES)
    ]
    results = run_bass_kernel_spmd(nc, in_maps, core_ids=list(range(NCORES)))
    act = _unshard(results.results, "act_out")
    raw = _unshard(results.results, "raw_out")
    return act, raw


# revision 18
# speedup vs baseline: 3.2586x; 1.1863x over previous
"""Trainium2 Bass kernel for nn_Chambers (6-tower MLP + coupled sigmoid recurrence).

Data-parallel over 8 NeuronCores: each core processes a 16384-sample shard in
16 chunks of 1024 samples. res tiles are PE-transposed (fp32, exact) into
[100, 1024] activation tiles; the transposes land in rotation PSUM tiles so
no dedicated transpose bank exists. The 4 MLP layers run as fp32r matmuls
(full PE rate at N=512) with chamber pairs packed into 128 partition rows via
shifted zero-padded stationaries. L3 pair 2 is double-packed across chunk
pairs (rows 0:64 = even chunk, 64:128 = odd chunk) so its silu runs once per
two chunks. L4 accumulates all 16 chunks into a persistent [96, 1024] PSUM
tile using sliding-window stationary bands whose nonzero columns land at rows
6i+c; the sigmoid recurrence and the raw output read that tile directly, so
no per-chunk raw copies exist.

Sync discipline: at most 1 sem wait + 1 update per engine instruction.
"Touch" ops (tiny matmul / 1-elem activation / 1-elem copy) pre-observe
cross-engine sems; PE touches write into the about-to-be-started psum tile
(the start=True matmul or transpose re-zeroes the cells), so no scratch psum
bank is needed. PSUM budget: 2 rotation tags x [128,1024] (4 banks) + pc
double-pack tile (2) + praw (2) = 8 banks.
"""
import numpy as np

import concourse.bass as bass
import concourse.mybir as mybir
from concourse.bass_utils import run_bass_kernel_spmd
from concourse.tile import TileContext
from concourse.tile_scheduler import N_PROCS
from concourse.vector_clock import ScopedClock
from bass_rust import add_dep_helper

F32 = mybir.dt.float32
F32R = mybir.dt.float32r
AF = mybir.ActivationFunctionType
ALU = mybir.AluOpType

B = 131072
NCORES = 8
BS = B // NCORES           # 16384 samples per core
T = 1024                   # chunk (samples)
NCH = BS // T              # 16 chunks
RES_DIM = 100
CF_ITERS = 5
CF_K = 0.02

# wf (fp32) column layout
IDC = 0        # identity [128,128] for PE transposes
B1C = 128      # 6 cols: b1 per chamber
B2PC = 134     # 3 cols: pair-packed b2 (rows 0:64 even, 64:128 odd)
B3PC = 137     # 1 col: merged pairs 0,1 b3 (rows 32c..32c+32 = b3[c])
B3P2 = 138     # 1 col: pair2 b3 (rows 0:32 b3[4], 32:64 b3[5])
B4C = 139      # 1 col: b4 tiled x16 over 96 rows
FCOLS = 140

# wr (fp32r) column layout
W1C = 0                    # 6*128
W2EC = 768                 # 3*64  (even chambers)
W2OC = 960                 # 3*128 (odd chambers shifted to out rows 64:127)
W3AC = 1344                # 128 (pair1 shifted to out rows 64:127)
W3BC0 = 1472               # 64  (pair0 -> out rows 0:63)
W3P2C = 1536               # 64  (pair2 -> out rows 0:63)
CDC = 1792                 # 96 (block-diag decay*coupling*k per 6-row group)
I96C = 1888                # 96 identity
W4AB = 1984                # 186-col sliding band: window +90-6i -> cols 6i+c
W4BB = 2170                # 186-col sliding band: window +90-6i -> 6i+4, 6i+5
RCOLS = 2360


class TC(TileContext):
    """TileContext with a walrus-compatible epilogue (split final waits)."""

    def _drain_and_barrier(self, tick_clock, wait_clock):
        nc = self.nc
        full = ScopedClock({None: tick_clock.global_clock})
        for scope, vc in full.items():
            for proc in range(N_PROCS):
                t = vc.peek_next(proc) - 1
                if t > 0:
                    sc = ScopedClock()
                    sc.require_at_least(scope, proc, t)
                    w = nc.sync.nop(nofuse=True)
                    wait_clock.add_sem_waits(w.ins, sc)
        for eng in nc.engines.values():
            eng.drain(fusable=False)
        nc.all_engine_barrier(sem_only=True)
        assert self.sems is not None
        popped = nc._tile_sem_poison_stack.pop()
        assert popped is self._sem_poison
        nc.clear_and_free_semaphores(list(self.sems.allocated().values()))
        for eng in nc.engines.values():
            eng.drain(fusable=False)
        nc.all_engine_barrier(sem_only=True)


def _order(after_inst, before_inst):
    if after_inst is not None and before_inst is not None:
        add_dep_helper(after_inst.ins, before_inst.ins, sync=False, reason="order")


def build_module():
    nc = bass.Bass()
    res_d = nc.dram_tensor("res", [BS, RES_DIM], F32, kind="ExternalInput")
    wf_d = nc.dram_tensor("wf", [128, FCOLS], F32, kind="ExternalInput")
    wr_d = nc.dram_tensor("wr", [128, RCOLS], F32R, kind="ExternalInput")
    raw_d = nc.dram_tensor("raw_out", [96, T], F32, kind="ExternalOutput")
    act_d = nc.dram_tensor("act_out", [96, T], F32, kind="ExternalOutput")

    with TC(nc) as tc:
        with (
            tc.tile_pool(name="wconst", bufs=1) as wpool,
            tc.tile_pool(name="sbrt", bufs=3) as sbrt,
            tc.tile_pool(name="sbh", bufs=2) as sbh,
            tc.tile_pool(name="sbrec", bufs=1) as sbrec,
            tc.tile_pool(name="psmm", bufs=3, space="PSUM") as psmm,
            tc.tile_pool(name="psraw", bufs=1, space="PSUM") as psraw,
        ):
            # DMA issue order: chunk-0 res and L1 weights first so compute
            # starts early; the bulky remainder streams behind.
            res_sb0 = wpool.tile([128, 8 * RES_DIM], F32)
            nc.sync.dma_start(
                out=res_sb0[:],
                in_=res_d[0:T].rearrange("(p n) d -> p (n d)", p=128))
            wf = wpool.tile([128, FCOLS], F32)
            nc.sync.dma_start(out=wf[:], in_=wf_d[:])
            wr = wpool.tile([128, RCOLS], F32R)
            nc.sync.dma_start(out=wr[:, 0:W2EC], in_=wr_d[:, 0:W2EC])
            nc.sync.dma_start(out=wr[:, W2EC:CDC], in_=wr_d[:, W2EC:CDC])
            res_sb1 = wpool.tile([128, 3 * 8 * RES_DIM], F32)
            nc.sync.dma_start(
                out=res_sb1[:],
                in_=res_d[T:4 * T].rearrange("(p n) d -> p (n d)", p=128))
            nc.sync.dma_start(out=wr[:, CDC:], in_=wr_d[:, CDC:])
            res_sb2 = wpool.tile([128, (NCH - 4) * 8 * RES_DIM], F32)
            nc.sync.dma_start(
                out=res_sb2[:],
                in_=res_d[4 * T:].rearrange("(p n) d -> p (n d)", p=128))
            ident = wf[:, IDC:IDC + 128]

            raw_sb = sbrec.tile([96, T], F32)
            act_r = sbrec.tile([96, T], F32R)
            act_o = sbrec.tile([96, T], F32)
            raw_r = sbrec.tile([96, T], F32R)
            scr = sbrec.tile([1, 4], F32)
            scrA = sbrec.tile([1, 512], F32)
            scrD = sbrec.tile([1, 64], F32)

            praw = psraw.tile([96, T], F32, name="praw")

            # ACT touch op: observe the wf DMA lane before first bias use
            nc.scalar.activation(scr[0:1, 0:1], wf[0:1, B1C:B1C + 1], AF.Copy)

            # Rolling state. Rule: each instruction carries at most one sem
            # wait; every cross-engine dependency is pre-observed by a real
            # "touch" instruction. PE touches write into the new psum tile's
            # first cells, which the subsequent start=True matmul (or
            # transpose) re-zeroes.
            pe_tail = None
            act_tail = None
            dve_tail = None
            acol = [0]
            dcol = [0]

            def pe_touch_into(dst_tile, cell, src_ap):
                nonlocal pe_tail
                m = nc.tensor.matmul(dst_tile[0:1, 2 * cell:2 * cell + 2],
                                     src_ap[:, 0:1], src_ap[:, 0:2],
                                     start=True, stop=True)
                _order(m, pe_tail)
                pe_tail = m
                return m

            def act_touch(src_ap):
                nonlocal act_tail
                t = acol[0]; acol[0] += 1
                assert t < 512
                s = nc.scalar.activation(scrA[0:1, t:t + 1], src_ap, AF.Copy)
                _order(s, act_tail)
                act_tail = s
                return s

            def dve_touch(src_ap):
                nonlocal dve_tail
                t = dcol[0]; dcol[0] += 1
                assert t < 64
                c = nc.vector.tensor_copy(scrD[0:1, t:t + 1], src_ap)
                _order(c, dve_tail)
                dve_tail = c
                return c

            tag_rr = [0]
            tag_state = [None, None, None]

            def new_mm_tile(name, extra_srcs=(), width=T):
                tg = tag_rr[0] % 3
                tag_rr[0] += 1
                t = psmm.tile([128, width], F32, tag=f"mm{tg}", bufs=1,
                              name=name)
                cell = 0
                st = tag_state[tg]
                if st is not None:
                    tile_, row_, col_ = st
                    pe_touch_into(t, cell, tile_[row_:row_ + 1, col_:col_ + 2])
                    cell += 1
                    tag_state[tg] = None
                for src in extra_srcs:
                    pe_touch_into(t, cell, src)
                    cell += 1
                return t, tg

            def mm(out_ap, lhs_ap, rhs_ap, **kw):
                nonlocal pe_tail
                m = nc.tensor.matmul(out_ap, lhs_ap, rhs_ap, **kw)
                _order(m, pe_tail)
                pe_tail = m
                return m

            def silu(out_ap, pm_ap, bias_ap, out_tile, tg, func=AF.Silu):
                nonlocal act_tail
                s = nc.scalar.activation(out_ap, pm_ap, func, bias=bias_ap)
                _order(s, act_tail)
                act_tail = s
                if tg is not None:
                    tag_state[tg] = (out_tile, 0, 0)
                return s

            rt_tiles = {}
            h3a_tiles = {}
            h3b_tiles = {}

            def emit_tr(i):
                """Transpose chunk i's res into a [100, T] f32r SBUF tile via
                a rotation psum tile."""
                nonlocal pe_tail, dve_tail
                if i == 0:
                    rq, coff = res_sb0, 0
                elif i < 4:
                    rq, coff = res_sb1, (i - 1) * 8 * RES_DIM
                else:
                    rq, coff = res_sb2, (i - 4) * 8 * RES_DIM
                extras = []
                if i in (0, 1, 4):
                    extras.append(rq[0:1, coff:coff + 2])
                if i == 0:
                    extras.append(wr[0:1, 0:2])
                ptr, tgp = new_mm_tile("ptr", extra_srcs=extras)
                for n in range(8):
                    t_ = nc.tensor.transpose(
                        ptr[0:100, n * 128:(n + 1) * 128],
                        rq[:, coff + n * RES_DIM:coff + (n + 1) * RES_DIM],
                        ident,
                    )
                    _order(t_, pe_tail)
                    pe_tail = t_
                rT = sbrt.tile([100, T], F32R, tag="rT", bufs=3, name="rT")
                dve_touch(ptr[0:1, 0:1])
                cp = nc.vector.tensor_copy(rT[:100, :], ptr[:100, :])
                _order(cp, dve_tail)
                dve_tail = cp
                tag_state[tgp] = (rT, 0, 0)
                rt_tiles[i] = rT

            def emit_l4a(j):
                # praw[6j+c] += W4[c] . h3a_j via sliding-window band
                h3a = h3a_tiles.pop(j)
                if j == 0:
                    # first praw write: touch cells land in the start region
                    # (the start=True matmul re-zeroes them); they observe
                    # the wr tail-segment DMA lane and the h3a silu. For j>0
                    # the matmul itself carries the single silu wait (or it
                    # is already observed via rotation touches).
                    pe_touch_into(praw, 0, wr[0:1, W4AB:W4AB + 2])
                    pe_touch_into(praw, 1, h3a[0:1, 0:2])
                st = wr[:, W4AB + 90 - 6 * j:W4AB + 186 - 6 * j]
                for s in range(2):
                    mm(praw[0:96, s * 512:(s + 1) * 512], st,
                       h3a[:, s * 512:(s + 1) * 512],
                       start=(j == 0), stop=False)

            def emit_l4b(j, last=False):
                h3b = h3b_tiles.pop(j)
                st = wr[0:64, W4BB + 90 - 6 * j:W4BB + 186 - 6 * j]
                for s in range(2):
                    mm(praw[0:96, s * 512:(s + 1) * 512], st,
                       h3b[:, s * 512:(s + 1) * 512],
                       start=False, stop=(last and s == 1))

            emit_tr(0)
            for i in range(NCH):
                rT = rt_tiles.pop(i)

                # L1: 3 chamber-pairs
                h1s = []
                for cp in range(3):
                    ha = sbh.tile([128, T], F32R, tag="h1", bufs=7, name="h1a")
                    hb = sbh.tile([128, T], F32R, tag="h1", bufs=7, name="h1b")
                    extras = [rT[0:1, 0:2]] if cp == 0 else []
                    pa, ta = new_mm_tile("pm1a", extra_srcs=extras)
                    pb, tb = new_mm_tile("pm1b")
                    for s in range(2):
                        mm(pa[:, s * 512:(s + 1) * 512],
                           wr[0:100, W1C + 2 * cp * 128:W1C + (2 * cp + 1) * 128],
                           rT[:, s * 512:(s + 1) * 512], start=True, stop=True)
                    for s in range(2):
                        mm(pb[:, s * 512:(s + 1) * 512],
                           wr[0:100, W1C + (2 * cp + 1) * 128:W1C + (2 * cp + 2) * 128],
                           rT[:, s * 512:(s + 1) * 512], start=True, stop=True)
                    act_touch(pb[0:1, 512:513])
                    silu(ha[:], pa[:], wf[:, B1C + 2 * cp:B1C + 2 * cp + 1],
                         ha, ta)
                    silu(hb[:], pb[:], wf[:, B1C + 2 * cp + 1:B1C + 2 * cp + 2],
                         hb, tb)
                    h1s.extend([ha, hb])
                    if cp == 1 and i >= 1:
                        emit_l4a(i - 1)
                        emit_l4b(i - 1)

                if i + 1 < NCH:
                    emit_tr(i + 1)

                # L2: per pair, odd chamber shifted to rows 64:127
                h2s = []
                l2t = []
                for pr in range(3):
                    extras = []
                    if i == 0 and pr == 0:
                        extras.append(wr[0:1, W2EC:W2EC + 2])
                    pm2, tg2 = new_mm_tile("pm2", extra_srcs=extras)
                    for s in range(2):
                        mm(pm2[:, s * 512:(s + 1) * 512],
                           wr[:, W2OC + pr * 128:W2OC + (pr + 1) * 128],
                           h1s[2 * pr + 1][:, s * 512:(s + 1) * 512],
                           start=True, stop=False)
                        mm(pm2[0:64, s * 512:(s + 1) * 512],
                           wr[:, W2EC + pr * 64:W2EC + (pr + 1) * 64],
                           h1s[2 * pr][:, s * 512:(s + 1) * 512],
                           start=False, stop=True)
                    l2t.append((pm2, tg2))
                for pr in range(3):
                    pm2, tg2 = l2t[pr]
                    if pr == 0:
                        act_touch(pm2[0:1, 512:513])
                    h2 = sbh.tile([128, T], F32R, tag="h2", bufs=4, name="h2")
                    silu(h2[:], pm2[:], wf[:, B2PC + pr:B2PC + pr + 1],
                         h2, tg2)
                    h2s.append(h2)

                # L3 pairs 0,1 merged into one tile; pair 2 separate
                h3a = sbh.tile([128, T], F32R, tag="h3a", bufs=2, name="h3a")
                pa3, ta3 = new_mm_tile("pm3")
                for s in range(2):
                    mm(pa3[:, s * 512:(s + 1) * 512],
                       wr[:, W3AC:W3AC + 128],
                       h2s[1][:, s * 512:(s + 1) * 512], start=True, stop=False)
                    mm(pa3[0:64, s * 512:(s + 1) * 512],
                       wr[:, W3BC0:W3BC0 + 64],
                       h2s[0][:, s * 512:(s + 1) * 512], start=False, stop=True)
                h3b = sbh.tile([64, T], F32R, tag="h3b", bufs=2, name="h3b")
                pc, tc_ = new_mm_tile("pmc")
                for s in range(2):
                    mm(pc[0:64, s * 512:(s + 1) * 512],
                       wr[:, W3P2C:W3P2C + 64],
                       h2s[2][:, s * 512:(s + 1) * 512], start=True, stop=True)
                act_touch(pc[0:1, 512:513])
                silu(h3a[:], pa3[:], wf[:, B3PC:B3PC + 1], h3a, ta3)
                silu(h3b[0:64, :], pc[0:64, :], wf[0:64, B3P2:B3P2 + 1],
                     h3b, tc_)
                h3a_tiles[i] = h3a
                h3b_tiles[i] = h3b

            # drain deferred tail work
            emit_l4a(NCH - 1)
            emit_l4b(NCH - 1, last=True)

            # ---- coupled sigmoid recurrence on praw [96, T] ----
            b4ap = wf[0:96, B4C:B4C + 1]
            act_touch(praw[0:1, 0:1])
            sig = nc.scalar.activation(act_r[:], praw[0:96, :], AF.Sigmoid,
                                       bias=b4ap)
            _order(sig, act_tail)
            act_tail = sig
            rb = nc.scalar.activation(raw_sb[:], praw[0:96, :], AF.Identity,
                                      bias=b4ap)
            _order(rb, act_tail)
            act_tail = rb
            nc.sync.dma_start(out=raw_d[:], in_=raw_sb[:])
            dve_touch(raw_sb[0:1, 0:1])
            cpr = nc.vector.tensor_copy(raw_r[:], raw_sb[:])
            _order(cpr, dve_tail)
            dve_tail = cpr

            for kk in range(CF_ITERS):
                dst = act_r if kk < CF_ITERS - 1 else act_o
                for s in range(2):
                    extras = []
                    if s == 0:
                        # observe the latest sigmoid (kk-1 s=1) and, on the
                        # first iter, the DVE raw_r copy
                        extras.append(act_r[0:1, 0:2])
                        if kk == 0:
                            extras.append(raw_r[0:1, 0:2])
                    pm5, tg5 = new_mm_tile("pm5", extra_srcs=extras,
                                           width=512)
                    mm(pm5[0:96, 0:512],
                       wr[0:96, CDC:CDC + 96],
                       act_r[:, s * 512:(s + 1) * 512],
                       start=True, stop=False)
                    mm(pm5[0:96, 0:512],
                       wr[0:96, I96C:I96C + 96],
                       raw_r[:, s * 512:(s + 1) * 512],
                       start=False, stop=True)
                    act_touch(pm5[0:1, 0:1])
                    sg = nc.scalar.activation(
                        dst[:, s * 512:(s + 1) * 512], pm5[0:96, 0:512],
                        AF.Sigmoid)
                    _order(sg, act_tail)
                    act_tail = sg
                    tag_state[tg5] = (dst, 0, s * 512)

            nc.sync.dma_start(out=act_d[:], in_=act_o[:])

    return nc


def _pack_consts(W1, b1, W2, b2, W3, b3, W4, b4, coupling, decay):
    wf = np.zeros((128, FCOLS), dtype=np.float32)
    wf[:, IDC:IDC + 128] = np.eye(128, dtype=np.float32)
    for c in range(6):
        wf[:, B1C + c] = b1[c]
    for pr in range(3):
        wf[0:64, B2PC + pr] = b2[2 * pr]
        wf[64:128, B2PC + pr] = b2[2 * pr + 1]
    for c in range(4):
        wf[c * 32:(c + 1) * 32, B3PC] = b3[c]
    wf[0:32, B3P2] = b3[4]
    wf[32:64, B3P2] = b3[5]
    wf[0:96, B4C] = np.tile(b4, NCH)

    wr = np.zeros((128, RCOLS), dtype=np.float32)
    for c in range(6):
        wr[0:100, W1C + c * 128:W1C + (c + 1) * 128] = W1[c]
    for pr in range(3):
        wr[:, W2EC + pr * 64:W2EC + (pr + 1) * 64] = W2[2 * pr]
        wr[:, W2OC + pr * 128 + 64:W2OC + (pr + 1) * 128] = W2[2 * pr + 1]
    # L3 merged pairs 0,1: pair1 shifted to out rows 64:127
    wr[0:64, W3AC + 64:W3AC + 96] = W3[2]
    wr[64:128, W3AC + 96:W3AC + 128] = W3[3]
    wr[0:64, W3BC0:W3BC0 + 32] = W3[0]
    wr[64:128, W3BC0 + 32:W3BC0 + 64] = W3[1]
    # L3 pair 2: out rows 0:63
    wr[0:64, W3P2C:W3P2C + 32] = W3[4]
    wr[64:128, W3P2C + 32:W3P2C + 64] = W3[5]
    cd = (decay[:, None] * coupling * CF_K).astype(np.float32)
    for g in range(16):
        wr[6 * g:6 * g + 6, CDC + 6 * g:CDC + 6 * g + 6] = cd
    wr[0:96, I96C:I96C + 96] = np.eye(96, dtype=np.float32)
    # W4 sliding bands: window j = band[:, 90-6j : 186-6j] -> out col 6j+c
    for c in range(4):
        wr[c * 32:(c + 1) * 32, W4AB + 90 + c] = W4[c]
    wr[0:32, W4BB + 94] = W4[4]
    wr[32:64, W4BB + 95] = W4[5]
    return wf, wr


def _unshard(per_core, key):
    """[96, T] layout (row 6i+c, col j) -> [BS, 6] per core, concat.

    Col j of chunk i: n = j//128, p = j%128 maps to sample i*T + 8p + n
    (p-major transpose block order)."""
    outs = []
    for r in per_core:
        a = r[key].reshape(NCH, 6, 8, 128)         # [i, c, n, p]
        out = a.transpose(0, 3, 2, 1).reshape(BS, 6)
        outs.append(out)
    return np.concatenate(outs, axis=0)


def kernel(res, W1, b1, W2, b2, W3, b3, W4, b4, coupling, decay):
    res = np.asarray(res, dtype=np.float32)
    args = [np.asarray(a, dtype=np.float32)
            for a in (W1, b1, W2, b2, W3, b3, W4, b4, coupling, decay)]
    wf, wr = _pack_consts(*args)

    nc = build_module()
    in_maps = [
        {"res": np.ascontiguousarray(res[i * BS:(i + 1) * BS]),
         "wf": wf, "wr": wr}
        for i in range(NCORES)
    ]
    results = run_bass_kernel_spmd(nc, in_maps, core_ids=list(range(NCORES)))
    act = _unshard(results.results, "act_out")
    raw = _unshard(results.results, "raw_out")
    return act, raw


# revision 19
# speedup vs baseline: 3.3405x; 1.0251x over previous
"""Trainium2 Bass kernel for nn_Chambers (6-tower MLP + coupled sigmoid recurrence).

Data-parallel over 8 NeuronCores: each core processes a 16384-sample shard in
16 chunks of 1024 samples. res tiles are PE-transposed (fp32, exact) into
[100, 1024] activation tiles; the transposes land in rotation PSUM tiles so
no dedicated transpose bank exists. The 4 MLP layers run as fp32r matmuls
(full PE rate at N=512) with chamber pairs packed into 128 partition rows via
shifted zero-padded stationaries. L3 pair 2 is double-packed across chunk
pairs (rows 0:64 = even chunk, 64:128 = odd chunk) so its silu runs once per
two chunks. L4 accumulates all 16 chunks into a persistent [96, 1024] PSUM
tile using sliding-window stationary bands whose nonzero columns land at rows
6i+c; the sigmoid recurrence and the raw output read that tile directly, so
no per-chunk raw copies exist.

Sync discipline: at most 1 sem wait + 1 update per engine instruction.
"Touch" ops (tiny matmul / 1-elem activation / 1-elem copy) pre-observe
cross-engine sems; PE touches write into the about-to-be-started psum tile
(the start=True matmul or transpose re-zeroes the cells), so no scratch psum
bank is needed. PSUM budget: 2 rotation tags x [128,1024] (4 banks) + pc
double-pack tile (2) + praw (2) = 8 banks.
"""
import numpy as np

import concourse.bass as bass
import concourse.mybir as mybir
from concourse.bass_utils import run_bass_kernel_spmd
from concourse.tile import TileContext
from concourse.tile_scheduler import N_PROCS
from concourse.vector_clock import ScopedClock
from bass_rust import add_dep_helper

F32 = mybir.dt.float32
F32R = mybir.dt.float32r
AF = mybir.ActivationFunctionType
ALU = mybir.AluOpType

B = 131072
NCORES = 8
BS = B // NCORES           # 16384 samples per core
T = 1024                   # chunk (samples)
NCH = BS // T              # 16 chunks
RES_DIM = 100
CF_ITERS = 5
CF_K = 0.02

# wf (fp32) column layout
IDC = 0        # identity [128,128] for PE transposes
B1C = 128      # 6 cols: b1 per chamber
B2PC = 134     # 3 cols: pair-packed b2 (rows 0:64 even, 64:128 odd)
B3PC = 137     # 1 col: merged pairs 0,1 b3 (rows 32c..32c+32 = b3[c])
B3P2 = 138     # 1 col: pair2 b3 (rows 0:32 b3[4], 32:64 b3[5])
B4C = 139      # 1 col: b4 tiled x16 over 96 rows
FCOLS = 140

# wr (fp32r) column layout
W1C = 0                    # 6*128
W2EC = 768                 # 3*64  (even chambers)
W2OC = 960                 # 3*128 (odd chambers shifted to out rows 64:127)
W3AC = 1344                # 128 (pair1 shifted to out rows 64:127)
W3BC0 = 1472               # 64  (pair0 -> out rows 0:63)
W3P2C = 1536               # 64  (pair2 -> out rows 0:63)
CDC = 1792                 # 96 (block-diag decay*coupling*k per 6-row group)
I96C = 1888                # 96 identity
W4AB = 1984                # 186-col sliding band: window +90-6i -> cols 6i+c
W4BB = 2170                # 186-col sliding band: window +90-6i -> 6i+4, 6i+5
RCOLS = 2360


class TC(TileContext):
    """TileContext with a walrus-compatible epilogue (split final waits)."""

    def _drain_and_barrier(self, tick_clock, wait_clock):
        nc = self.nc
        full = ScopedClock({None: tick_clock.global_clock})
        for scope, vc in full.items():
            for proc in range(N_PROCS):
                t = vc.peek_next(proc) - 1
                if t > 0:
                    sc = ScopedClock()
                    sc.require_at_least(scope, proc, t)
                    w = nc.sync.nop(nofuse=True)
                    wait_clock.add_sem_waits(w.ins, sc)
        for eng in nc.engines.values():
            eng.drain(fusable=False)
        nc.all_engine_barrier(sem_only=True)
        assert self.sems is not None
        popped = nc._tile_sem_poison_stack.pop()
        assert popped is self._sem_poison
        nc.clear_and_free_semaphores(list(self.sems.allocated().values()))
        for eng in nc.engines.values():
            eng.drain(fusable=False)
        nc.all_engine_barrier(sem_only=True)


def _order(after_inst, before_inst):
    if after_inst is not None and before_inst is not None:
        add_dep_helper(after_inst.ins, before_inst.ins, sync=False, reason="order")


def build_module():
    nc = bass.Bass()
    res_d = nc.dram_tensor("res", [BS, RES_DIM], F32, kind="ExternalInput")
    wf_d = nc.dram_tensor("wf", [128, FCOLS], F32, kind="ExternalInput")
    wr_d = nc.dram_tensor("wr", [128, RCOLS], F32R, kind="ExternalInput")
    raw_d = nc.dram_tensor("raw_out", [96, T], F32, kind="ExternalOutput")
    act_d = nc.dram_tensor("act_out", [96, T], F32, kind="ExternalOutput")

    with TC(nc) as tc:
        with (
            tc.tile_pool(name="wconst", bufs=1) as wpool,
            tc.tile_pool(name="sbrt", bufs=3) as sbrt,
            tc.tile_pool(name="sbh", bufs=2) as sbh,
            tc.tile_pool(name="sbrec", bufs=1) as sbrec,
            tc.tile_pool(name="psmm", bufs=3, space="PSUM") as psmm,
            tc.tile_pool(name="psraw", bufs=1, space="PSUM") as psraw,
        ):
            # DMA issue order: chunk-0 res and L1 weights first so compute
            # starts early; the bulky remainder streams behind.
            res_sb0 = wpool.tile([128, 8 * RES_DIM], F32)
            nc.sync.dma_start(
                out=res_sb0[:],
                in_=res_d[0:T].rearrange("(p n) d -> p (n d)", p=128))
            wf = wpool.tile([128, FCOLS], F32)
            nc.sync.dma_start(out=wf[:], in_=wf_d[:])
            wr = wpool.tile([128, RCOLS], F32R)
            nc.sync.dma_start(out=wr[:, 0:W2EC], in_=wr_d[:, 0:W2EC])
            nc.sync.dma_start(out=wr[:, W2EC:CDC], in_=wr_d[:, W2EC:CDC])
            res_sb1 = wpool.tile([128, 3 * 8 * RES_DIM], F32)
            nc.sync.dma_start(
                out=res_sb1[:],
                in_=res_d[T:4 * T].rearrange("(p n) d -> p (n d)", p=128))
            nc.sync.dma_start(out=wr[:, CDC:], in_=wr_d[:, CDC:])
            res_sb2 = wpool.tile([128, (NCH - 4) * 8 * RES_DIM], F32)
            nc.sync.dma_start(
                out=res_sb2[:],
                in_=res_d[4 * T:].rearrange("(p n) d -> p (n d)", p=128))
            ident = wf[:, IDC:IDC + 128]

            raw_sb = sbrec.tile([96, T], F32)
            act_r = sbrec.tile([96, T], F32R)
            act_o = sbrec.tile([96, T], F32)
            raw_r = sbrec.tile([96, T], F32R)
            scr = sbrec.tile([1, 4], F32)
            scrA = sbrec.tile([1, 512], F32)
            scrD = sbrec.tile([1, 64], F32)

            praw = psraw.tile([96, T], F32, name="praw")

            # ACT touch op: observe the wf DMA lane before first bias use
            nc.scalar.activation(scr[0:1, 0:1], wf[0:1, B1C:B1C + 1], AF.Copy)

            # Rolling state. Rule: each instruction carries at most one sem
            # wait; every cross-engine dependency is pre-observed by a real
            # "touch" instruction. PE touches write into the new psum tile's
            # first cells, which the subsequent start=True matmul (or
            # transpose) re-zeroes.
            pe_tail = None
            act_tail = None
            dve_tail = None
            acol = [0]
            dcol = [0]

            def pe_touch_into(dst_tile, cell, src_ap):
                nonlocal pe_tail
                m = nc.tensor.matmul(dst_tile[0:1, 2 * cell:2 * cell + 2],
                                     src_ap[:, 0:1], src_ap[:, 0:2],
                                     start=True, stop=True)
                _order(m, pe_tail)
                pe_tail = m
                return m

            def act_touch(src_ap):
                nonlocal act_tail
                t = acol[0]; acol[0] += 1
                assert t < 512
                s = nc.scalar.activation(scrA[0:1, t:t + 1], src_ap, AF.Copy)
                _order(s, act_tail)
                act_tail = s
                return s

            def dve_touch(src_ap):
                nonlocal dve_tail
                t = dcol[0]; dcol[0] += 1
                assert t < 64
                c = nc.vector.tensor_copy(scrD[0:1, t:t + 1], src_ap)
                _order(c, dve_tail)
                dve_tail = c
                return c

            tag_rr = [0]
            tag_state = [None, None, None]

            def new_mm_tile(name, extra_srcs=(), width=T):
                tg = tag_rr[0] % 3
                tag_rr[0] += 1
                t = psmm.tile([128, width], F32, tag=f"mm{tg}", bufs=1,
                              name=name)
                cell = 0
                st = tag_state[tg]
                if st is not None:
                    tile_, row_, col_ = st
                    pe_touch_into(t, cell, tile_[row_:row_ + 1, col_:col_ + 2])
                    cell += 1
                    tag_state[tg] = None
                for src in extra_srcs:
                    pe_touch_into(t, cell, src)
                    cell += 1
                return t, tg

            def mm(out_ap, lhs_ap, rhs_ap, **kw):
                nonlocal pe_tail
                m = nc.tensor.matmul(out_ap, lhs_ap, rhs_ap, **kw)
                _order(m, pe_tail)
                pe_tail = m
                return m

            def silu(out_ap, pm_ap, bias_ap, out_tile, tg, func=AF.Silu):
                nonlocal act_tail
                s = nc.scalar.activation(out_ap, pm_ap, func, bias=bias_ap)
                _order(s, act_tail)
                act_tail = s
                if tg is not None:
                    tag_state[tg] = (out_tile, 0, 0)
                return s

            rt_tiles = {}
            h3a_tiles = {}
            h3b_tiles = {}

            def emit_tr(i):
                """Transpose chunk i's res into a [100, T] f32r SBUF tile via
                a rotation psum tile."""
                nonlocal pe_tail, dve_tail
                if i == 0:
                    rq, coff = res_sb0, 0
                elif i < 4:
                    rq, coff = res_sb1, (i - 1) * 8 * RES_DIM
                else:
                    rq, coff = res_sb2, (i - 4) * 8 * RES_DIM
                extras = []
                if i in (0, 1, 4):
                    extras.append(rq[0:1, coff:coff + 2])
                if i == 0:
                    extras.append(wr[0:1, 0:2])
                ptr, tgp = new_mm_tile("ptr", extra_srcs=extras)
                for n in range(8):
                    t_ = nc.tensor.transpose(
                        ptr[0:100, n * 128:(n + 1) * 128],
                        rq[:, coff + n * RES_DIM:coff + (n + 1) * RES_DIM],
                        ident,
                    )
                    _order(t_, pe_tail)
                    pe_tail = t_
                rT = sbrt.tile([100, T], F32R, tag="rT", bufs=3, name="rT")
                dve_touch(ptr[0:1, 0:1])
                cp = nc.vector.tensor_copy(rT[:100, :], ptr[:100, :])
                _order(cp, dve_tail)
                dve_tail = cp
                tag_state[tgp] = (rT, 0, 0)
                rt_tiles[i] = rT

            def emit_l4a(j):
                # praw[6j+c] += W4[c] . h3a_j via sliding-window band
                h3a = h3a_tiles.pop(j)
                if j == 0:
                    # first praw write: touch cells land in the start region
                    # (the start=True matmul re-zeroes them); they observe
                    # the wr tail-segment DMA lane and the h3a silu. For j>0
                    # the matmul itself carries the single silu wait (or it
                    # is already observed via rotation touches).
                    pe_touch_into(praw, 0, wr[0:1, W4AB:W4AB + 2])
                    pe_touch_into(praw, 1, h3a[0:1, 0:2])
                st = wr[:, W4AB + 90 - 6 * j:W4AB + 186 - 6 * j]
                for s in range(2):
                    mm(praw[0:96, s * 512:(s + 1) * 512], st,
                       h3a[:, s * 512:(s + 1) * 512],
                       start=(j == 0), stop=False)

            def emit_l4b(j, last=False):
                h3b = h3b_tiles.pop(j)
                st = wr[0:64, W4BB + 90 - 6 * j:W4BB + 186 - 6 * j]
                for s in range(2):
                    mm(praw[0:96, s * 512:(s + 1) * 512], st,
                       h3b[:, s * 512:(s + 1) * 512],
                       start=False, stop=(last and s == 1))

            emit_tr(0)
            for i in range(NCH):
                rT = rt_tiles.pop(i)

                # L1: 3 chamber-pairs
                h1s = []
                for cp in range(3):
                    ha = sbh.tile([128, T], F32R, tag="h1", bufs=7, name="h1a")
                    hb = sbh.tile([128, T], F32R, tag="h1", bufs=7, name="h1b")
                    extras = [rT[0:1, 0:2]] if cp == 0 else []
                    pa, ta = new_mm_tile("pm1a", extra_srcs=extras)
                    pb, tb = new_mm_tile("pm1b")
                    for s in range(2):
                        mm(pa[:, s * 512:(s + 1) * 512],
                           wr[0:100, W1C + 2 * cp * 128:W1C + (2 * cp + 1) * 128],
                           rT[:, s * 512:(s + 1) * 512], start=True, stop=True)
                    for s in range(2):
                        mm(pb[:, s * 512:(s + 1) * 512],
                           wr[0:100, W1C + (2 * cp + 1) * 128:W1C + (2 * cp + 2) * 128],
                           rT[:, s * 512:(s + 1) * 512], start=True, stop=True)
                    act_touch(pb[0:1, 512:513])
                    silu(ha[:], pa[:], wf[:, B1C + 2 * cp:B1C + 2 * cp + 1],
                         ha, ta)
                    silu(hb[:], pb[:], wf[:, B1C + 2 * cp + 1:B1C + 2 * cp + 2],
                         hb, tb)
                    h1s.extend([ha, hb])
                    if cp == 1 and i >= 1:
                        emit_l4a(i - 1)
                        emit_l4b(i - 1)

                if i + 1 < NCH:
                    emit_tr(i + 1)

                # L2: per pair, odd chamber shifted to rows 64:127
                h2s = []
                l2t = []
                for pr in range(3):
                    extras = []
                    if i == 0 and pr == 0:
                        extras.append(wr[0:1, W2EC:W2EC + 2])
                    pm2, tg2 = new_mm_tile("pm2", extra_srcs=extras)
                    for s in range(2):
                        mm(pm2[:, s * 512:(s + 1) * 512],
                           wr[:, W2OC + pr * 128:W2OC + (pr + 1) * 128],
                           h1s[2 * pr + 1][:, s * 512:(s + 1) * 512],
                           start=True, stop=False)
                        mm(pm2[0:64, s * 512:(s + 1) * 512],
                           wr[:, W2EC + pr * 64:W2EC + (pr + 1) * 64],
                           h1s[2 * pr][:, s * 512:(s + 1) * 512],
                           start=False, stop=True)
                    l2t.append((pm2, tg2))
                for pr in range(3):
                    pm2, tg2 = l2t[pr]
                    if pr == 0:
                        act_touch(pm2[0:1, 512:513])
                    h2 = sbh.tile([128, T], F32R, tag="h2", bufs=4, name="h2")
                    silu(h2[:], pm2[:], wf[:, B2PC + pr:B2PC + pr + 1],
                         h2, tg2)
                    h2s.append(h2)

                # L3 pairs 0,1 merged into one tile; pair 2 separate
                h3a = sbh.tile([128, T], F32R, tag="h3a", bufs=2, name="h3a")
                pa3, ta3 = new_mm_tile("pm3")
                for s in range(2):
                    mm(pa3[:, s * 512:(s + 1) * 512],
                       wr[:, W3AC:W3AC + 128],
                       h2s[1][:, s * 512:(s + 1) * 512], start=True, stop=False)
                    mm(pa3[0:64, s * 512:(s + 1) * 512],
                       wr[:, W3BC0:W3BC0 + 64],
                       h2s[0][:, s * 512:(s + 1) * 512], start=False, stop=True)
                h3b = sbh.tile([64, T], F32R, tag="h3b", bufs=2, name="h3b")
                pc, tc_ = new_mm_tile("pmc")
                for s in range(2):
                    mm(pc[0:64, s * 512:(s + 1) * 512],
                       wr[:, W3P2C:W3P2C + 64],
                       h2s[2][:, s * 512:(s + 1) * 512], start=True, stop=True)
                # silu(h3a) only needs pa3 (ready during silu(pm2_2)); keep it
                # ahead of the pc-dependent touch so ACT never idles here.
                act_touch(pa3[0:1, 512:513])
                silu(h3a[:], pa3[:], wf[:, B3PC:B3PC + 1], h3a, ta3)
                act_touch(pc[0:1, 512:513])
                silu(h3b[0:64, :], pc[0:64, :], wf[0:64, B3P2:B3P2 + 1],
                     h3b, tc_)
                h3a_tiles[i] = h3a
                h3b_tiles[i] = h3b

            # drain deferred tail work
            emit_l4a(NCH - 1)
            emit_l4b(NCH - 1, last=True)

            # ---- coupled sigmoid recurrence on praw [96, T] ----
            b4ap = wf[0:96, B4C:B4C + 1]
            act_touch(praw[0:1, 0:1])
            sig = nc.scalar.activation(act_r[:], praw[0:96, :], AF.Sigmoid,
                                       bias=b4ap)
            _order(sig, act_tail)
            act_tail = sig
            rb = nc.scalar.activation(raw_sb[:], praw[0:96, :], AF.Identity,
                                      bias=b4ap)
            _order(rb, act_tail)
            act_tail = rb
            nc.sync.dma_start(out=raw_d[:], in_=raw_sb[:])
            dve_touch(raw_sb[0:1, 0:1])
            cpr = nc.vector.tensor_copy(raw_r[:], raw_sb[:])
            _order(cpr, dve_tail)
            dve_tail = cpr

            for kk in range(CF_ITERS):
                dst = act_r if kk < CF_ITERS - 1 else act_o
                for s in range(2):
                    extras = []
                    if s == 0:
                        # observe the latest sigmoid (kk-1 s=1) and, on the
                        # first iter, the DVE raw_r copy
                        extras.append(act_r[0:1, 0:2])
                        if kk == 0:
                            extras.append(raw_r[0:1, 0:2])
                    pm5, tg5 = new_mm_tile("pm5", extra_srcs=extras,
                                           width=512)
                    mm(pm5[0:96, 0:512],
                       wr[0:96, CDC:CDC + 96],
                       act_r[:, s * 512:(s + 1) * 512],
                       start=True, stop=False)
                    mm(pm5[0:96, 0:512],
                       wr[0:96, I96C:I96C + 96],
                       raw_r[:, s * 512:(s + 1) * 512],
                       start=False, stop=True)
                    act_touch(pm5[0:1, 0:1])
                    sg = nc.scalar.activation(
                        dst[:, s * 512:(s + 1) * 512], pm5[0:96, 0:512],
                        AF.Sigmoid)
                    _order(sg, act_tail)
                    act_tail = sg
                    tag_state[tg5] = (dst, 0, s * 512)

            nc.sync.dma_start(out=act_d[:], in_=act_o[:])

    return nc


def _pack_consts(W1, b1, W2, b2, W3, b3, W4, b4, coupling, decay):
    wf = np.zeros((128, FCOLS), dtype=np.float32)
    wf[:, IDC:IDC + 128] = np.eye(128, dtype=np.float32)
    for c in range(6):
        wf[:, B1C + c] = b1[c]
    for pr in range(3):
        wf[0:64, B2PC + pr] = b2[2 * pr]
        wf[64:128, B2PC + pr] = b2[2 * pr + 1]
    for c in range(4):
        wf[c * 32:(c + 1) * 32, B3PC] = b3[c]
    wf[0:32, B3P2] = b3[4]
    wf[32:64, B3P2] = b3[5]
    wf[0:96, B4C] = np.tile(b4, NCH)

    wr = np.zeros((128, RCOLS), dtype=np.float32)
    for c in range(6):
        wr[0:100, W1C + c * 128:W1C + (c + 1) * 128] = W1[c]
    for pr in range(3):
        wr[:, W2EC + pr * 64:W2EC + (pr + 1) * 64] = W2[2 * pr]
        wr[:, W2OC + pr * 128 + 64:W2OC + (pr + 1) * 128] = W2[2 * pr + 1]
    # L3 merged pairs 0,1: pair1 shifted to out rows 64:127
    wr[0:64, W3AC + 64:W3AC + 96] = W3[2]
    wr[64:128, W3AC + 96:W3AC + 128] = W3[3]
    wr[0:64, W3BC0:W3BC0 + 32] = W3[0]
    wr[64:128, W3BC0 + 32:W3BC0 + 64] = W3[1]
    # L3 pair 2: out rows 0:63
    wr[0:64, W3P2C:W3P2C + 32] = W3[4]
    wr[64:128, W3P2C + 32:W3P2C + 64] = W3[5]
    cd = (decay[:, None] * coupling * CF_K).astype(np.float32)
    for g in range(16):
        wr[6 * g:6 * g + 6, CDC + 6 * g:CDC + 6 * g + 6] = cd
    wr[0:96, I96C:I96C + 96] = np.eye(96, dtype=np.float32)
    # W4 sliding bands: window j = band[:, 90-6j : 186-6j] -> out col 6j+c
    for c in range(4):
        wr[c * 32:(c + 1) * 32, W4AB + 90 + c] = W4[c]
    wr[0:32, W4BB + 94] = W4[4]
    wr[32:64, W4BB + 95] = W4[5]
    return wf, wr


def _unshard(per_core, key):
    """[96, T] layout (row 6i+c, col j) -> [BS, 6] per core, concat.

    Col j of chunk i: n = j//128, p = j%128 maps to sample i*T + 8p + n
    (p-major transpose block order)."""
    outs = []
    for r in per_core:
        a = r[key].reshape(NCH, 6, 8, 128)         # [i, c, n, p]
        out = a.transpose(0, 3, 2, 1).reshape(BS, 6)
        outs.append(out)
    return np.concatenate(outs, axis=0)


def kernel(res, W1, b1, W2, b2, W3, b3, W4, b4, coupling, decay):
    res = np.asarray(res, dtype=np.float32)
    args = [np.asarray(a, dtype=np.float32)
            for a in (W1, b1, W2, b2, W3, b3, W4, b4, coupling, decay)]
    wf, wr = _pack_consts(*args)

    nc = build_module()
    in_maps = [
        {"res": np.ascontiguousarray(res[i * BS:(i + 1) * BS]),
         "wf": wf, "wr": wr}
        for i in range(NCORES)
    ]
    results = run_bass_kernel_spmd(nc, in_maps, core_ids=list(range(NCORES)))
    act = _unshard(results.results, "act_out")
    raw = _unshard(results.results, "raw_out")
    return act, raw


# revision 24
# speedup vs baseline: 3.3699x; 1.0088x over previous
"""Trainium2 Bass kernel for nn_Chambers (6-tower MLP + coupled sigmoid recurrence).

Data-parallel over 8 NeuronCores: each core processes a 16384-sample shard in
16 chunks of 1024 samples. res tiles are PE-transposed (fp32, exact) into
[100, 1024] activation tiles; the transposes land in rotation PSUM tiles so
no dedicated transpose bank exists. The 4 MLP layers run as fp32r matmuls
(full PE rate at N=512) with chamber pairs packed into 128 partition rows via
shifted zero-padded stationaries. L3 pair 2 is double-packed across chunk
pairs (rows 0:64 = even chunk, 64:128 = odd chunk) so its silu runs once per
two chunks. L4 accumulates all 16 chunks into a persistent [96, 1024] PSUM
tile using sliding-window stationary bands whose nonzero columns land at rows
6i+c; the sigmoid recurrence and the raw output read that tile directly, so
no per-chunk raw copies exist.

Sync discipline: at most 1 sem wait + 1 update per engine instruction.
"Touch" ops (tiny matmul / 1-elem activation / 1-elem copy) pre-observe
cross-engine sems; PE touches write into the about-to-be-started psum tile
(the start=True matmul or transpose re-zeroes the cells), so no scratch psum
bank is needed. PSUM budget: 2 rotation tags x [128,1024] (4 banks) + pc
double-pack tile (2) + praw (2) = 8 banks.
"""
import numpy as np

import concourse.bass as bass
import concourse.mybir as mybir
from concourse.bass_utils import run_bass_kernel_spmd
from concourse.tile import TileContext
from concourse.tile_scheduler import N_PROCS
from concourse.vector_clock import ScopedClock
from bass_rust import add_dep_helper

F32 = mybir.dt.float32
F32R = mybir.dt.float32r
AF = mybir.ActivationFunctionType
ALU = mybir.AluOpType

B = 131072
NCORES = 8
BS = B // NCORES           # 16384 samples per core
T = 1024                   # chunk (samples)
NCH = BS // T              # 16 chunks
RES_DIM = 100
CF_ITERS = 5
CF_K = 0.02

# wf (fp32) column layout
IDC = 0        # identity [128,128] for PE transposes
B1C = 128      # 6 cols: b1 per chamber
B2PC = 134     # 3 cols: pair-packed b2 (rows 0:64 even, 64:128 odd)
B3PC = 137     # 1 col: merged pairs 0,1 b3 (rows 32c..32c+32 = b3[c])
B3P2 = 138     # 1 col: pair2 b3 (rows 0:32 b3[4], 32:64 b3[5])
B4C = 139      # 1 col: b4 tiled x16 over 96 rows
FCOLS = 140

# wr (fp32r) column layout
W1C = 0                    # 6*128
W2EC = 768                 # 3*64  (even chambers)
W2OC = 960                 # 3*128 (odd chambers shifted to out rows 64:127)
W3AC = 1344                # 128 (pair1 shifted to out rows 64:127)
W3BC0 = 1472               # 64  (pair0 -> out rows 0:63)
W3P2C = 1536               # 64  (pair2 -> out rows 0:63)
CDC = 1792                 # 96 (block-diag decay*coupling*k per 6-row group)
I96C = 1888                # 96 identity
W4AB = 1984                # 186-col sliding band: window +90-6i -> cols 6i+c
W4BB = 2170                # 186-col sliding band: window +90-6i -> 6i+4, 6i+5
RCOLS = 2360


class TC(TileContext):
    """TileContext with a walrus-compatible epilogue (split final waits)."""

    def _drain_and_barrier(self, tick_clock, wait_clock):
        nc = self.nc
        full = ScopedClock({None: tick_clock.global_clock})
        for scope, vc in full.items():
            for proc in range(N_PROCS):
                t = vc.peek_next(proc) - 1
                if t > 0:
                    sc = ScopedClock()
                    sc.require_at_least(scope, proc, t)
                    w = nc.sync.nop(nofuse=True)
                    wait_clock.add_sem_waits(w.ins, sc)
        for eng in nc.engines.values():
            eng.drain(fusable=False)
        nc.all_engine_barrier(sem_only=True)
        assert self.sems is not None
        popped = nc._tile_sem_poison_stack.pop()
        assert popped is self._sem_poison
        nc.clear_and_free_semaphores(list(self.sems.allocated().values()))
        for eng in nc.engines.values():
            eng.drain(fusable=False)
        nc.all_engine_barrier(sem_only=True)


def _order(after_inst, before_inst):
    if after_inst is not None and before_inst is not None:
        add_dep_helper(after_inst.ins, before_inst.ins, sync=False, reason="order")


def build_module():
    nc = bass.Bass()
    res_d = nc.dram_tensor("res", [BS, RES_DIM], F32, kind="ExternalInput")
    wf_d = nc.dram_tensor("wf", [128, FCOLS], F32, kind="ExternalInput")
    wr_d = nc.dram_tensor("wr", [128, RCOLS], F32R, kind="ExternalInput")
    raw_d = nc.dram_tensor("raw_out", [96, T], F32, kind="ExternalOutput")
    act_d = nc.dram_tensor("act_out", [96, T], F32, kind="ExternalOutput")

    with TC(nc) as tc:
        with (
            tc.tile_pool(name="wconst", bufs=1) as wpool,
            tc.tile_pool(name="sbrt", bufs=3) as sbrt,
            tc.tile_pool(name="sbh", bufs=2) as sbh,
            tc.tile_pool(name="sbrec", bufs=1) as sbrec,
            tc.tile_pool(name="psmm", bufs=3, space="PSUM") as psmm,
            tc.tile_pool(name="psraw", bufs=1, space="PSUM") as psraw,
        ):
            # DMA issue order: chunk-0 res and L1 weights first so compute
            # starts early; the bulky remainder streams behind.
            res_sb0 = wpool.tile([128, 8 * RES_DIM], F32)
            nc.sync.dma_start(
                out=res_sb0[:],
                in_=res_d[0:T].rearrange("(p n) d -> p (n d)", p=128))
            wr = wpool.tile([128, RCOLS], F32R)
            nc.sync.dma_start(out=wr[:, 0:256], in_=wr_d[:, 0:256])
            wf = wpool.tile([128, FCOLS], F32)
            nc.sync.dma_start(out=wf[:], in_=wf_d[:])
            nc.sync.dma_start(out=wr[:, 256:CDC], in_=wr_d[:, 256:CDC])
            res_sb1 = wpool.tile([128, 3 * 8 * RES_DIM], F32)
            nc.sync.dma_start(
                out=res_sb1[:],
                in_=res_d[T:4 * T].rearrange("(p n) d -> p (n d)", p=128))
            nc.sync.dma_start(out=wr[:, CDC:], in_=wr_d[:, CDC:])
            res_sb2 = wpool.tile([128, (NCH - 4) * 8 * RES_DIM], F32)
            nc.sync.dma_start(
                out=res_sb2[:],
                in_=res_d[4 * T:].rearrange("(p n) d -> p (n d)", p=128))
            ident = wf[:, IDC:IDC + 128]

            # Recurrence state is split into independent s=0/1 half tiles so
            # the two 512-col chains never serialize on tile-granular deps.
            raw_sb = [sbrec.tile([96, 512], F32, name=f"raw_sb{s}")
                      for s in range(2)]
            act_rh = [sbrec.tile([96, 512], F32R, name=f"act_r{s}")
                      for s in range(2)]
            act_oh = [sbrec.tile([96, 512], F32, name=f"act_o{s}")
                      for s in range(2)]
            raw_rh = [sbrec.tile([96, 512], F32R, name=f"raw_r{s}")
                      for s in range(2)]
            scr = sbrec.tile([1, 4], F32)
            scrA = sbrec.tile([1, 512], F32)
            scrD = sbrec.tile([1, 64], F32)

            praw = [psraw.tile([96, 512], F32, name=f"praw{s}")
                    for s in range(2)]

            # ACT touch op: observe the wf DMA lane before first bias use
            nc.scalar.activation(scr[0:1, 0:1], wf[0:1, B1C:B1C + 1], AF.Copy)

            # Rolling state. Rule: each instruction carries at most one sem
            # wait; every cross-engine dependency is pre-observed by a real
            # "touch" instruction. PE touches write into the new psum tile's
            # first cells, which the subsequent start=True matmul (or
            # transpose) re-zeroes.
            pe_tail = None
            act_tail = None
            dve_tail = None
            acol = [0]
            dcol = [0]

            def pe_touch_into(dst_tile, cell, src_ap):
                nonlocal pe_tail
                m = nc.tensor.matmul(dst_tile[0:1, 2 * cell:2 * cell + 2],
                                     src_ap[:, 0:1], src_ap[:, 0:2],
                                     start=True, stop=True)
                _order(m, pe_tail)
                pe_tail = m
                return m

            def act_touch(src_ap):
                nonlocal act_tail
                t = acol[0]; acol[0] += 1
                assert t < 512
                s = nc.scalar.activation(scrA[0:1, t:t + 1], src_ap, AF.Copy)
                _order(s, act_tail)
                act_tail = s
                return s

            def dve_touch(src_ap):
                nonlocal dve_tail
                t = dcol[0]; dcol[0] += 1
                assert t < 64
                c = nc.vector.tensor_copy(scrD[0:1, t:t + 1], src_ap)
                _order(c, dve_tail)
                dve_tail = c
                return c

            tag_rr = [0]
            tag_state = [None, None, None]

            def new_mm_tile(name, extra_srcs=(), width=T):
                tg = tag_rr[0] % 3
                tag_rr[0] += 1
                t = psmm.tile([128, width], F32, tag=f"mm{tg}", bufs=1,
                              name=name)
                cell = 0
                st = tag_state[tg]
                if st is not None:
                    tile_, row_, col_ = st
                    pe_touch_into(t, cell, tile_[row_:row_ + 1, col_:col_ + 2])
                    cell += 1
                    tag_state[tg] = None
                for src in extra_srcs:
                    pe_touch_into(t, cell, src)
                    cell += 1
                return t, tg

            def mm(out_ap, lhs_ap, rhs_ap, **kw):
                nonlocal pe_tail
                m = nc.tensor.matmul(out_ap, lhs_ap, rhs_ap, **kw)
                _order(m, pe_tail)
                pe_tail = m
                return m

            def silu(out_ap, pm_ap, bias_ap, out_tile, tg, func=AF.Silu):
                nonlocal act_tail
                s = nc.scalar.activation(out_ap, pm_ap, func, bias=bias_ap)
                _order(s, act_tail)
                act_tail = s
                if tg is not None:
                    tag_state[tg] = (out_tile, 0, 0)
                return s

            rt_tiles = {}
            h3a_tiles = {}
            h3b_tiles = {}

            def emit_tr(i):
                """Transpose chunk i's res into a [100, T] f32r SBUF tile via
                a rotation psum tile."""
                nonlocal pe_tail, dve_tail
                if i == 0:
                    rq, coff = res_sb0, 0
                elif i < 4:
                    rq, coff = res_sb1, (i - 1) * 8 * RES_DIM
                else:
                    rq, coff = res_sb2, (i - 4) * 8 * RES_DIM
                extras = []
                if i in (0, 1, 4):
                    extras.append(rq[0:1, coff:coff + 2])
                if i == 0:
                    extras.append(wr[0:1, 0:2])
                ptr, tgp = new_mm_tile("ptr", extra_srcs=extras)
                for n in range(8):
                    t_ = nc.tensor.transpose(
                        ptr[0:100, n * 128:(n + 1) * 128],
                        rq[:, coff + n * RES_DIM:coff + (n + 1) * RES_DIM],
                        ident,
                    )
                    _order(t_, pe_tail)
                    pe_tail = t_
                rT = sbrt.tile([100, T], F32R, tag="rT", bufs=3, name="rT")
                dve_touch(ptr[0:1, 0:1])
                cp = nc.vector.tensor_copy(rT[:100, :], ptr[:100, :])
                _order(cp, dve_tail)
                dve_tail = cp
                tag_state[tgp] = (rT, 0, 0)
                rt_tiles[i] = rT

            def emit_l4a(j):
                # praw[6j+c] += W4[c] . h3a_j via sliding-window band
                h3a = h3a_tiles.pop(j)
                if j == 0:
                    # first praw write: touch cells land in the start region
                    # (the start=True matmul re-zeroes them); they observe
                    # the wr tail-segment DMA lane and the h3a silu. For j>0
                    # the matmul itself carries the single silu wait (or it
                    # is already observed via rotation touches).
                    pe_touch_into(praw[0], 0, wr[0:1, W4AB:W4AB + 2])
                    pe_touch_into(praw[0], 1, h3a[0:1, 0:2])
                st = wr[:, W4AB + 90 - 6 * j:W4AB + 186 - 6 * j]
                for s in range(2):
                    mm(praw[s][0:96, 0:512], st,
                       h3a[:, s * 512:(s + 1) * 512],
                       start=(j == 0), stop=False)

            def emit_l4b(j, last=False):
                h3b = h3b_tiles.pop(j)
                st = wr[0:64, W4BB + 90 - 6 * j:W4BB + 186 - 6 * j]
                for s in range(2):
                    mm(praw[s][0:96, 0:512], st,
                       h3b[:, s * 512:(s + 1) * 512],
                       start=False, stop=last)

            emit_tr(0)
            for i in range(NCH):
                rT = rt_tiles.pop(i)

                # L1: 3 chamber-pairs
                h1s = []
                for cp in range(3):
                    ha = sbh.tile([128, T], F32R, tag="h1", bufs=7, name="h1a")
                    hb = sbh.tile([128, T], F32R, tag="h1", bufs=7, name="h1b")
                    extras = [rT[0:1, 0:2]] if cp == 0 else []
                    if i == 0 and cp == 1:
                        extras.append(wr[0:1, 256:258])
                    pa, ta = new_mm_tile("pm1a", extra_srcs=extras)
                    pb, tb = new_mm_tile("pm1b")
                    for s in range(2):
                        mm(pa[:, s * 512:(s + 1) * 512],
                           wr[0:100, W1C + 2 * cp * 128:W1C + (2 * cp + 1) * 128],
                           rT[:, s * 512:(s + 1) * 512], start=True, stop=True)
                    for s in range(2):
                        mm(pb[:, s * 512:(s + 1) * 512],
                           wr[0:100, W1C + (2 * cp + 1) * 128:W1C + (2 * cp + 2) * 128],
                           rT[:, s * 512:(s + 1) * 512], start=True, stop=True)
                    act_touch(pb[0:1, 512:513])
                    silu(ha[:], pa[:], wf[:, B1C + 2 * cp:B1C + 2 * cp + 1],
                         ha, ta)
                    silu(hb[:], pb[:], wf[:, B1C + 2 * cp + 1:B1C + 2 * cp + 2],
                         hb, tb)
                    h1s.extend([ha, hb])
                    if cp == 1 and i >= 1:
                        emit_l4a(i - 1)
                        emit_l4b(i - 1)

                if i + 1 < NCH:
                    emit_tr(i + 1)

                # L2: per pair, odd chamber shifted to rows 64:127
                h2s = []
                l2t = []
                for pr in range(3):
                    extras = []
                    if i == 0 and pr == 0:
                        extras.append(wr[0:1, W2EC:W2EC + 2])
                    pm2, tg2 = new_mm_tile("pm2", extra_srcs=extras)
                    for s in range(2):
                        mm(pm2[:, s * 512:(s + 1) * 512],
                           wr[:, W2OC + pr * 128:W2OC + (pr + 1) * 128],
                           h1s[2 * pr + 1][:, s * 512:(s + 1) * 512],
                           start=True, stop=False)
                        mm(pm2[0:64, s * 512:(s + 1) * 512],
                           wr[:, W2EC + pr * 64:W2EC + (pr + 1) * 64],
                           h1s[2 * pr][:, s * 512:(s + 1) * 512],
                           start=False, stop=True)
                    l2t.append((pm2, tg2))
                for pr in range(3):
                    pm2, tg2 = l2t[pr]
                    if pr == 0:
                        act_touch(pm2[0:1, 512:513])
                    h2 = sbh.tile([128, T], F32R, tag="h2", bufs=4, name="h2")
                    silu(h2[:], pm2[:], wf[:, B2PC + pr:B2PC + pr + 1],
                         h2, tg2)
                    h2s.append(h2)

                # L3 pairs 0,1 merged into one tile; pair 2 separate
                h3a = sbh.tile([128, T], F32R, tag="h3a", bufs=2, name="h3a")
                pa3, ta3 = new_mm_tile("pm3")
                for s in range(2):
                    mm(pa3[:, s * 512:(s + 1) * 512],
                       wr[:, W3AC:W3AC + 128],
                       h2s[1][:, s * 512:(s + 1) * 512], start=True, stop=False)
                    mm(pa3[0:64, s * 512:(s + 1) * 512],
                       wr[:, W3BC0:W3BC0 + 64],
                       h2s[0][:, s * 512:(s + 1) * 512], start=False, stop=True)
                h3b = sbh.tile([64, T], F32R, tag="h3b", bufs=2, name="h3b")
                pc, tc_ = new_mm_tile("pmc")
                for s in range(2):
                    mm(pc[0:64, s * 512:(s + 1) * 512],
                       wr[:, W3P2C:W3P2C + 64],
                       h2s[2][:, s * 512:(s + 1) * 512], start=True, stop=True)
                # silu(h3a) only needs pa3 (ready during silu(pm2_2)); keep it
                # ahead of the pc-dependent touch so ACT never idles here.
                act_touch(pa3[0:1, 512:513])
                silu(h3a[:], pa3[:], wf[:, B3PC:B3PC + 1], h3a, ta3)
                h3a_tiles[i] = h3a
                if i == NCH - 1:
                    emit_l4a(i)
                act_touch(pc[0:1, 512:513])
                silu(h3b[0:64, :], pc[0:64, :], wf[0:64, B3P2:B3P2 + 1],
                     h3b, tc_)
                h3b_tiles[i] = h3b
                if i == NCH - 1:
                    emit_l4b(i, last=True)

            # ---- coupled sigmoid recurrence on praw halves [96, 512] ----
            b4ap = wf[0:96, B4C:B4C + 1]
            for s in range(2):
                act_touch(praw[s][0:1, 0:1])
                sig = nc.scalar.activation(act_rh[s][:], praw[s][0:96, :],
                                           AF.Sigmoid, bias=b4ap)
                _order(sig, act_tail)
                act_tail = sig
                rb = nc.scalar.activation(raw_sb[s][:], praw[s][0:96, :],
                                          AF.Identity, bias=b4ap)
                _order(rb, act_tail)
                act_tail = rb
                nc.sync.dma_start(out=raw_d[:, s * 512:(s + 1) * 512],
                                  in_=raw_sb[s][:])
                dve_touch(raw_sb[s][0:1, 0:1])
                cpr = nc.vector.tensor_copy(raw_rh[s][:], raw_sb[s][:])
                _order(cpr, dve_tail)
                dve_tail = cpr

            for kk in range(CF_ITERS):
                for s in range(2):
                    dst = act_rh[s] if kk < CF_ITERS - 1 else act_oh[s]
                    extras = [act_rh[s][0:1, 0:2]]
                    if kk == 0:
                        extras.append(raw_rh[s][0:1, 0:2])
                    pm5, tg5 = new_mm_tile("pm5", extra_srcs=extras,
                                           width=512)
                    mm(pm5[0:96, 0:512],
                       wr[0:96, CDC:CDC + 96],
                       act_rh[s][:],
                       start=True, stop=False)
                    mm(pm5[0:96, 0:512],
                       wr[0:96, I96C:I96C + 96],
                       raw_rh[s][:],
                       start=False, stop=True)
                    act_touch(pm5[0:1, 0:1])
                    sg = nc.scalar.activation(dst[:], pm5[0:96, 0:512],
                                              AF.Sigmoid)
                    _order(sg, act_tail)
                    act_tail = sg
                    tag_state[tg5] = (dst, 0, 0)
                    if kk == CF_ITERS - 1:
                        nc.sync.dma_start(out=act_d[:, s * 512:(s + 1) * 512],
                                          in_=act_oh[s][:])

    return nc


def _pack_consts(W1, b1, W2, b2, W3, b3, W4, b4, coupling, decay):
    wf = np.zeros((128, FCOLS), dtype=np.float32)
    wf[:, IDC:IDC + 128] = np.eye(128, dtype=np.float32)
    for c in range(6):
        wf[:, B1C + c] = b1[c]
    for pr in range(3):
        wf[0:64, B2PC + pr] = b2[2 * pr]
        wf[64:128, B2PC + pr] = b2[2 * pr + 1]
    for c in range(4):
        wf[c * 32:(c + 1) * 32, B3PC] = b3[c]
    wf[0:32, B3P2] = b3[4]
    wf[32:64, B3P2] = b3[5]
    wf[0:96, B4C] = np.tile(b4, NCH)

    wr = np.zeros((128, RCOLS), dtype=np.float32)
    for c in range(6):
        wr[0:100, W1C + c * 128:W1C + (c + 1) * 128] = W1[c]
    for pr in range(3):
        wr[:, W2EC + pr * 64:W2EC + (pr + 1) * 64] = W2[2 * pr]
        wr[:, W2OC + pr * 128 + 64:W2OC + (pr + 1) * 128] = W2[2 * pr + 1]
    # L3 merged pairs 0,1: pair1 shifted to out rows 64:127
    wr[0:64, W3AC + 64:W3AC + 96] = W3[2]
    wr[64:128, W3AC + 96:W3AC + 128] = W3[3]
    wr[0:64, W3BC0:W3BC0 + 32] = W3[0]
    wr[64:128, W3BC0 + 32:W3BC0 + 64] = W3[1]
    # L3 pair 2: out rows 0:63
    wr[0:64, W3P2C:W3P2C + 32] = W3[4]
    wr[64:128, W3P2C + 32:W3P2C + 64] = W3[5]
    cd = (decay[:, None] * coupling * CF_K).astype(np.float32)
    for g in range(16):
        wr[6 * g:6 * g + 6, CDC + 6 * g:CDC + 6 * g + 6] = cd
    wr[0:96, I96C:I96C + 96] = np.eye(96, dtype=np.float32)
    # W4 sliding bands: window j = band[:, 90-6j : 186-6j] -> out col 6j+c
    for c in range(4):
        wr[c * 32:(c + 1) * 32, W4AB + 90 + c] = W4[c]
    wr[0:32, W4BB + 94] = W4[4]
    wr[32:64, W4BB + 95] = W4[5]
    return wf, wr


def _unshard(per_core, key):
    """[96, T] layout (row 6i+c, col j) -> [BS, 6] per core, concat.

    Col j of chunk i: n = j//128, p = j%128 maps to sample i*T + 8p + n
    (p-major transpose block order)."""
    outs = []
    for r in per_core:
        a = r[key].reshape(NCH, 6, 8, 128)         # [i, c, n, p]
        out = a.transpose(0, 3, 2, 1).reshape(BS, 6)
        outs.append(out)
    return np.concatenate(outs, axis=0)


def kernel(res, W1, b1, W2, b2, W3, b3, W4, b4, coupling, decay):
    res = np.asarray(res, dtype=np.float32)
    args = [np.asarray(a, dtype=np.float32)
            for a in (W1, b1, W2, b2, W3, b3, W4, b4, coupling, decay)]
    wf, wr = _pack_consts(*args)

    nc = build_module()
    in_maps = [
        {"res": np.ascontiguousarray(res[i * BS:(i + 1) * BS]),
         "wf": wf, "wr": wr}
        for i in range(NCORES)
    ]
    results = run_bass_kernel_spmd(nc, in_maps, core_ids=list(range(NCORES)))
    act = _unshard(results.results, "act_out")
    raw = _unshard(results.results, "raw_out")
    return act, raw
